# revision 55
# baseline (speedup 1.0000x reference)
"""Trainium2 Bass kernel for LoRA-fused QKV + RoPE + GQA causal attention + o_proj.

Problem (hardcoded): B=2, S=2048, H=2048, NH=16, KVH=4, HD=128, R=16.

Sharding: 8 cores = batch(2) x kv-head-group(4). Core c handles batch b=c//4,
kv head g=c%4 (q heads 4g..4g+3). Each core computes its 4 heads' attention and
a partial o_proj ([S,H] partial over its 512 o-dims); host sums 4 partials per
batch.

v2 design vs the fp32r baseline:
- All matmuls in bf16 (1 cycle/row like fp32r, but FWL weight loads and half
  the DMA/SBUF). PSUM accumulation stays fp32. LoRA is folded into W on the
  host (W_eff = W^T + scale*A@B), biases are applied by the scalar engine
  during PSUM evacuation (activation bias=AP per-partition column).
- Everything stays in "transposed space": projections produce qT/kT/vT [d, s],
  scoresT [ks, qs] feeds AV directly, o_proj consumes outT [d, s] stationary.
- Softmax: no max-subtraction; exp on the scalar engine over PAIRS of score
  tiles (one ACTIVATE per 2 PSUM banks halves the ~310ns/instr overhead);
  column sums via an all-ones stationary matmul into PSUM; normalization on
  DVE (reciprocal_approx_fast + gpsimd partition_broadcast + multiply).
- Causal mask applied multiplicatively as exp(mask) with SKIP tiles dropped;
  the diagonal-tile patterns are generated on device as paired bf16 tiles.
- Single fused loop over 4 s-chunks; o_proj of chunk j flushes at the start of
  chunk j+1; attention is software-pipelined 2 score-pairs deep across head
  boundaries so PE never waits on exp.
- Weights resident in SBUF (loaded once); x/cos/sin double-buffered and
  prefetched a chunk ahead; initial DMAs split across queues so the first
  matmul starts ~2us in; output tiles stream back round-robin on 4 queues.
"""

import hashlib
import numpy as np
import ml_dtypes

import concourse.bass as bass
import concourse.mybir as mybir
import concourse.tile as tile
from concourse import bacc
from concourse.bass_utils import run_bass_kernel_spmd

B, S, H = 2, 2048, 2048
NH, KVH, HD = 16, 4, 128
R = 16
LORA_SCALE = 32.0 / 16.0
ATTN_SCALE = HD ** -0.5

NCORES = 8
GQ = NH // KVH          # 4 q heads per core
NT = GQ + 2             # 6 projection tiles: 4 q heads, 1 k, 1 v
QD = GQ * HD            # 512
CH = 512                # s-chunk width
NCH = S // CH           # 4 s-chunks
KT = H // 128           # 16 contraction k-tiles
NKS = S // 128          # 16 ks tiles
F32 = mybir.dt.float32
F32R = mybir.dt.float32r
BF16 = mybir.dt.bfloat16
NPBF16 = ml_dtypes.bfloat16

# tile classification codes (host-computed from exp(mask) tiles)
SKIP, PLAIN, MASKED = 0, 1, 2

# content tag: force a fresh NEFF cache key whenever this file changes
with open(__file__, "rb") as _f:
    KTAG = hashlib.sha1(_f.read()).hexdigest()[:10]
K_TAG_INT = int(KTAG, 16)


def _build(cls_grid, causal):
    """Build the SPMD program. cls_grid[i][j] in {SKIP, PLAIN, MASKED} for
    scoresT tile (ks_tile i, qs_chunk j). causal=True generates the diagonal
    mask tiles on device (no emaskT input)."""
    nc = bacc.Bacc("TRN2", target_bir_lowering=False)

    # host-packed for contiguous per-partition DMA:
    # x_pre[c, p, kt, s'] = x[b][s = c*CH+s', h = kt*128+p]  (bf16)
    xT = nc.dram_tensor("xT", [NCH, 128, KT, CH], BF16, kind="ExternalInput")
    # w_pre[p, t, kt, o] = w_eff[h = kt*128+p, t*128+o]  (bf16, LoRA folded)
    wT = nc.dram_tensor("wT", [128, NT, KT, 128], BF16, kind="ExternalInput")
    # [:, 0:NT] plain bias columns; [:, NT:2*NT] partition-swapped (rotate-half)
    biasT = nc.dram_tensor("biasT", [128, 2 * NT], F32, kind="ExternalInput")
    # cache-buster: the PJRT NEFF cache hashes the HLO minus backend_config
    DL = (K_TAG_INT % 97) + 1
    dummy = nc.dram_tensor("cachetag", [1, DL], F32, kind="ExternalInput")
    cosT = nc.dram_tensor("cosT", [HD, S], BF16, kind="ExternalInput")
    ssT = nc.dram_tensor("ssT", [HD, S], BF16, kind="ExternalInput")
    any_masked = any(cls_grid[i][j] == MASKED for i in range(NKS) for j in range(NCH))
    emaskT = None
    if not causal and any_masked:
        emaskT = nc.dram_tensor("emaskT", [S, S], BF16, kind="ExternalInput")
    owT = nc.dram_tensor("owT", [QD, H], BF16, kind="ExternalInput")
    out_p = nc.dram_tensor("out_p", [S, H], BF16, kind="ExternalOutput")

    live_per_j = [[i for i in range(NKS) if cls_grid[i][jj] != SKIP]
                  for jj in range(NCH)]
    masked_per_j = [[i for i in range(NKS) if cls_grid[i][jj] == MASKED]
                    for jj in range(NCH)]
    need = [max(jj, max(live_per_j[jj]) // (CH // 128)) for jj in range(NCH)]
    QCH_BUFS = max(2, max(need[jj] - jj for jj in range(NCH)) + 1)

    with tile.TileContext(nc) as tc:
        from concourse.masks import make_identity
        with tc.tile_pool(name="consts", bufs=1) as consts, \
             tc.tile_pool(name="persist", bufs=1) as persist, \
             tc.tile_pool(name="qch", bufs=QCH_BUFS) as qch_pool, \
             tc.tile_pool(name="outp", bufs=2) as outp_pool, \
             tc.tile_pool(name="p1", bufs=5) as p1, \
             tc.tile_pool(name="xch", bufs=2) as xch_pool, \
             tc.tile_pool(name="att", bufs=4) as att_pool, \
             tc.tile_pool(name="stgp", bufs=8) as stgp, \
             tc.tile_pool(name="fin", bufs=2) as fin, \
             tc.tile_pool(name="pp_pair", bufs=2, space="PSUM") as pp_pair, \
             tc.tile_pool(name="pp_o", bufs=2, space="PSUM") as pp_o, \
             tc.tile_pool(name="pp_sum", bufs=1, space="PSUM") as pp_sum, \
             tc.tile_pool(name="pp_t", bufs=1, space="PSUM") as pp_t:

            # ---- chunk-0 inputs first: x pieces split across queues ----
            x_tiles = {}

            def emit_x_dma(c, fine=False):
                x_c = xch_pool.tile([128, KT, CH], BF16, tag="x_c", name=f"x_{c}")
                x_tiles[c] = x_c
                if fine:
                    # 8 two-kt pieces: quick first matmul, low issue overhead
                    for kp in range(KT // 2):
                        q = nc.sync if kp % 2 == 0 else nc.gpsimd
                        q.dma_start(out=x_c[:, bass.ds(kp * 2, 2), :],
                                    in_=xT[c, :, bass.ds(kp * 2, 2), :])
                else:
                    qs = [nc.sync, nc.gpsimd, nc.sync, nc.gpsimd]
                    for kq in range(4):
                        qs[kq].dma_start(out=x_c[:, bass.ds(kq * 4, 4), :],
                                         in_=xT[c, :, bass.ds(kq * 4, 4), :])

            cs_tiles = {}

            def emit_cs_dma(c):
                sl = bass.ds(c * CH, CH)
                cos_c = xch_pool.tile([128, CH], BF16, tag="cos_c", name=f"cos_{c}")
                nc.sync.dma_start(out=cos_c, in_=cosT[:, sl])
                ss_c = xch_pool.tile([128, CH], BF16, tag="ss_c", name=f"ss_{c}")
                nc.sync.dma_start(out=ss_c, in_=ssT[:, sl])
                cs_tiles[c] = (cos_c, ss_c)

            # ---- weights resident, all on the scalar queue in t-need order;
            # x chunk 0 finely split on sync/gpsimd in parallel ----
            w_sb = persist.tile([128, NT, KT, 128], BF16, tag="w_sb")
            # t0 in 4 fine pieces so the first matmul starts after ~0.16MB
            for kq in range(4):
                nc.scalar.dma_start(out=w_sb[:, 0, bass.ds(kq * 4, 4), :],
                                    in_=wT[:, 0, bass.ds(kq * 4, 4), :])
            emit_x_dma(0, fine=True)
            for t in [1, GQ, 2, 3, NT - 1]:   # proj t-need order
                nc.scalar.dma_start(out=w_sb[:, t, :, :], in_=wT[:, t, :, :])
            emit_cs_dma(0)
            bias_sb = consts.tile([128, 2 * NT], F32, tag="bias_sb")
            nc.gpsimd.dma_start(out=bias_sb, in_=biasT[:, :])
            dummy_sb = consts.tile([1, 128], F32, tag="dummy_sb")
            nc.gpsimd.dma_start(out=dummy_sb[:, 0:DL], in_=dummy[:, :])

            # ---- small constants ----
            # full 128-col all-ones stationary: the denominator matmul then
            # broadcasts the column sums across all 128 PSUM partitions (no
            # gpsimd partition_broadcast needed) and keeps LDWEIGHTS
            # pull-ahead working (no col_grp restriction)
            ones_mat = consts.tile([128, 128], BF16, tag="ones_mat")
            nc.vector.memset(ones_mat, 1.0)
            ident_f = consts.tile([128, 128], F32, tag="ident_f")
            make_identity(nc, ident_f)
            identb = consts.tile([128, 128], BF16, tag="identb")
            nc.vector.tensor_copy(out=identb, in_=ident_f)

            # half-rotation permutation: perm[p, q] = 1 iff q == (p+64)%128.
            # Used as a matmul stationary to compute rotate-half on the PE
            # (swap via DMA would cross partitions on a busy queue).
            perm_f = consts.tile([128, 128], F32, tag="perm_f")
            nc.gpsimd.memset(perm_f, 0.0)
            for base in (64, -64):
                nc.gpsimd.affine_select(
                    out=perm_f, in_=perm_f,
                    compare_op=mybir.AluOpType.not_equal,
                    fill=1.0,
                    base=base,
                    channel_multiplier=1,
                    pattern=[[-1, 128]],
                )
            permb = consts.tile([128, 128], BF16, tag="permb")
            nc.vector.tensor_copy(out=permb, in_=perm_f)

            # causal diagonal mask tiles, paired: pattern depends on d = i - 4j
            # m_d[p, q] = 1.0 if p - q + 128*d <= 0 else 0.0
            diag_pairs = []
            if causal:
                scratch = consts.tile([128, CH], F32, tag="diag_scratch")
                for pp in range(2):
                    mp = consts.tile([128, 2, CH], BF16, tag=f"diagp{pp}",
                                     name=f"diagp{pp}")
                    for half in range(2):
                        dd = 2 * pp + half
                        nc.gpsimd.memset(scratch, 0.0)
                        nc.gpsimd.affine_select(
                            out=scratch, in_=scratch,
                            compare_op=mybir.AluOpType.is_gt,
                            fill=1.0,
                            base=128 * dd,
                            channel_multiplier=1,
                            pattern=[[-1, CH]],
                        )
                        nc.vector.tensor_copy(out=mp[:, half, :], in_=scratch)
                    diag_pairs.append(mp)

            # ow and next-chunk inputs are prefetched lazily (first attention)
            # so they don't steal startup fabric bandwidth from x(0)/w
            ow_sb = persist.tile([128, GQ, H], BF16, tag="ow_sb")
            ow_done = [False]

            def emit_prefetch(c):
                if c + 1 < NCH:
                    emit_x_dma(c + 1)
                    emit_cs_dma(c + 1)
                if not ow_done[0]:
                    ow_done[0] = True
                    nc.gpsimd.dma_start(
                        out=ow_sb, in_=owT.rearrange("(g p) n -> p g n", p=128))

            # ---- persistent tiles ----
            kT_full = persist.tile([128, S], BF16, tag="kT_full")
            v_nat = persist.tile([128, NKS, 128], BF16, tag="v_nat")  # [ks, tile, d]

            out_dma_q = [nc.sync, nc.gpsimd]
            out_dma_n = [0]

            def emit_oproj(args, final=False):
                cc, outT_ch = args
                qlist = [nc.sync, nc.gpsimd, nc.scalar] if final else [nc.gpsimd, nc.scalar]
                for st4 in range(CH // 128):
                    ssl = bass.ds(st4 * 128, 128)
                    dsl = bass.ds((cc * (CH // 128) + st4) * 128, 128)
                    for nch in range(NCH):
                        pop_deferred_v()
                        nsl = bass.ds(nch * CH, CH)
                        ps3 = pp_pair.tile([128, 2, CH], F32, tag="pair", name="ps3")
                        g = st4 * NCH + nch
                        half = g % 2
                        for h in range(GQ):
                            nc.tensor.matmul(ps3[:, half, :], outT_ch[h][:, ssl],
                                             ow_sb[:, h, nsl],
                                             start=(h == 0), stop=(h == GQ - 1))
                        stg = stgp.tile([128, CH], BF16, tag="stg")
                        if g % 2 == 0:
                            nc.vector.tensor_copy(out=stg, in_=ps3[:, half, :])
                        else:
                            nc.scalar.activation(out=stg, in_=ps3[:, half, :],
                                                 func=mybir.ActivationFunctionType.Copy)
                        q = qlist[out_dma_n[0] % len(qlist)]
                        out_dma_n[0] += 1
                        q.dma_start(out=out_p[dsl, nsl], in_=stg)

            q_chunks = {}
            deferred_v = []

            def pop_deferred_v():
                """Emit ONE pending v transpose (or nothing)."""
                if not deferred_v:
                    return
                c, vT_c, i4 = deferred_v.pop(0)
                i = c * (CH // 128) + i4
                ps_t = pp_t.tile([128, 128], BF16, tag="ps_t", name="ps_t")
                nc.tensor.transpose(ps_t, vT_c[:, bass.ds(i4 * 128, 128)], identb)
                nc.vector.tensor_copy(out=v_nat[:, i, :], in_=ps_t)

            def flush_deferred_v():
                while deferred_v:
                    pop_deferred_v()

            def emit_proj(c):
                x_c = x_tiles[c]
                cos_c, ss_c = cs_tiles[c]
                sl = bass.ds(c * CH, CH)

                q_ch = [qch_pool.tile([128, CH], BF16, tag=f"qch{h}", name=f"qch{h}_{c}")
                        for h in range(GQ)]
                q_chunks[c] = q_ch

                pending_rope = []

                def flush_rope():
                    # rotate-half via PE permutation matmul, then rope on DVE:
                    # dst = (raw+b)*cos + (perm@raw + swap(b))*ss
                    for t, raw in pending_rope:
                        bias_col = bias_sb[:, t:t + 1]
                        bias_sw = bias_sb[:, NT + t:NT + t + 1]
                        ps_sw = pp_t.tile([128, CH], F32, tag="ps_t", name="ps_sw")
                        nc.tensor.matmul(ps_sw, permb, raw, start=True, stop=True)
                        sw = p1.tile([128, CH], BF16, tag="sw")
                        nc.vector.scalar_tensor_tensor(
                            out=sw, in0=ps_sw, scalar=bias_sw, in1=ss_c,
                            op0=mybir.AluOpType.add, op1=mybir.AluOpType.mult)
                        dst = q_ch[t] if t < GQ else kT_full[:, sl]
                        nc.vector.scalar_tensor_tensor(
                            out=dst, in0=raw, scalar=bias_col, in1=cos_c,
                            op0=mybir.AluOpType.add, op1=mybir.AluOpType.mult)
                        nc.vector.tensor_add(dst, dst, sw)
                    pending_rope.clear()

                # t order: q0, q1, k, q2, q3, v; each tile's rope flushes one
                # MM-group later so the PSUM evacuation copy is never waited on
                t_list = [0, 1, GQ, 2, 3, NT - 1]
                for n, t in enumerate(t_list):
                    ps = pp_pair.tile([128, 2, CH], F32, tag="pair", name="ps_p")
                    half = n % 2
                    for kt in range(KT):
                        nc.tensor.matmul(ps[:, half, :], w_sb[:, t, kt, :],
                                         x_c[:, kt, :],
                                         start=(kt == 0), stop=(kt == KT - 1))
                    if t == NT - 1:   # v (bias folded on host); transpose deferred
                        vT_c = p1.tile([128, CH], BF16, tag="vT_c")
                        nc.scalar.activation(out=vT_c, in_=ps[:, half, :],
                                             func=mybir.ActivationFunctionType.Copy)
                        for i4 in range(CH // 128):
                            deferred_v.append((c, vT_c, i4))
                    else:
                        raw = p1.tile([128, CH], BF16, tag="raw")
                        nc.scalar.activation(out=raw, in_=ps[:, half, :],
                                             func=mybir.ActivationFunctionType.Copy)
                        flush_rope()
                        pending_rope.append((t, raw))
                flush_rope()

            def emit_attention(j):
                """Emits the attention for chunk j with a 2-pair software
                pipeline across head boundaries."""
                flush_deferred_v()
                sl = bass.ds(j * CH, CH)
                live = live_per_j[j]
                masked = set(masked_per_j[j])
                # build pair list: (i0, i1 or None)
                pairs = []
                for n in range(0, len(live), 2):
                    i0 = live[n]
                    i1 = live[n + 1] if n + 1 < len(live) else None
                    pairs.append((i0, i1))

                outT_ch = [outp_pool.tile([128, CH], BF16, tag=f"outT{h}",
                                          name=f"outT{h}_{j}") for h in range(GQ)]
                q_ch = q_chunks[j]

                # global pipeline over (head, pair)
                work = [(h, p) for h in range(GQ) for p in pairs]
                NP = len(work)
                attns = {}

                def emit_qk_exp(n):
                    h, (i0, i1) = work[n]
                    qh = q_ch[h]
                    pr = pp_pair.tile([128, 2, CH], F32, tag="pair", name="pr")
                    nc.tensor.matmul(pr[:, 0, :], kT_full[:, bass.ds(i0 * 128, 128)],
                                     qh, start=True, stop=True)
                    if i1 is not None:
                        nc.tensor.matmul(pr[:, 1, :],
                                         kT_full[:, bass.ds(i1 * 128, 128)],
                                         qh, start=True, stop=True)
                    attn = att_pool.tile([128, 2, CH], BF16, tag="attn")
                    src = pr if i1 is not None else pr[:, 0:1, :]
                    dst = attn if i1 is not None else attn[:, 0:1, :]
                    nc.scalar.activation(out=dst, in_=src,
                                         func=mybir.ActivationFunctionType.Exp,
                                         scale=float(ATTN_SCALE))
                    pair_masked = (i0 in masked) or (i1 in masked)
                    if pair_masked:
                        if causal:
                            # diag tiles are i = 4j + d; pairs aligned (d0,d1),(d2,d3)
                            dd = i0 - 4 * j
                            nc.vector.tensor_mul(attn, attn, diag_pairs[dd // 2])
                        else:
                            mt = att_pool.tile([128, 2, CH], BF16, tag="m_tile",
                                               bufs=3, name=f"mt_{j}_{h}_{i0}")
                            nc.gpsimd.dma_start(out=mt[:, 0, :],
                                                in_=emaskT[bass.ds(i0 * 128, 128), sl])
                            if i1 is not None:
                                nc.gpsimd.dma_start(
                                    out=mt[:, 1, :],
                                    in_=emaskT[bass.ds(i1 * 128, 128), sl])
                            msl = attn if i1 is not None else attn[:, 0:1, :]
                            mm = mt if i1 is not None else mt[:, 0:1, :]
                            nc.vector.tensor_mul(msl, msl, mm)
                    attns[n] = attn

                def emit_av(n):
                    h, (i0, i1) = work[n]
                    attn = attns.pop(n)
                    pidx = n % len(pairs)
                    first, last = (pidx == 0), (pidx == len(pairs) - 1)
                    ps_o = st_o[h]
                    ps_sum = st_sum[h]
                    nc.tensor.matmul(ps_o, v_nat[:, i0, :], attn[:, 0, :],
                                     start=first, stop=(last and i1 is None))
                    nc.tensor.matmul(ps_sum, ones_mat, attn[:, 0, :],
                                     start=first, stop=(last and i1 is None))
                    if i1 is not None:
                        nc.tensor.matmul(ps_o, v_nat[:, i1, :], attn[:, 1, :],
                                         start=False, stop=last)
                        nc.tensor.matmul(ps_sum, ones_mat, attn[:, 1, :],
                                         start=False, stop=last)
                    if last:
                        finalize(h)

                st_o, st_sum = {}, {}

                def start_head(h):
                    st_o[h] = pp_o.tile([128, CH], F32, tag="ps_o", name="ps_o")
                    st_sum[h] = pp_sum.tile([128, CH], F32, tag="ps_sum",
                                            name="ps_sum")

                def finalize(h):
                    # ps_sum already holds the denominator in every partition
                    recip = fin.tile([128, CH], F32, tag="recip")
                    nc.vector.reciprocal_approx_fast(out=recip, in_=st_sum[h])
                    nc.vector.tensor_mul(outT_ch[h], st_o[h], recip)

                PIPE = 2
                npairs = len(pairs)
                for n in range(NP):
                    if n % npairs == 0:
                        start_head(work[n][0])
                    emit_qk_exp(n)
                    if n >= PIPE:
                        emit_av(n - PIPE)
                for n in range(max(0, NP - PIPE), NP):
                    emit_av(n)
                return outT_ch

            # ---- main fused loop: proj(c) -> o_proj(c-1) flush (outT finalize
            # gets proj-length slack) -> x(c+1) prefetch -> attention(c) ----
            pending_oproj = None
            for c in range(NCH):
                emit_proj(c)
                if pending_oproj is not None:
                    emit_oproj(pending_oproj)
                    pending_oproj = None
                first_att = True
                for j in range(NCH):
                    if need[j] == c:
                        if pending_oproj is not None:
                            emit_oproj(pending_oproj)
                            pending_oproj = None
                        if first_att:
                            # prefetch next chunk's x/cos/sin during attention
                            emit_prefetch(c)
                            first_att = False
                        pending_oproj = (j, emit_attention(j))
                if first_att:
                    emit_prefetch(c)

            if pending_oproj is not None:
                emit_oproj(pending_oproj, final=True)
                pending_oproj = None

    nc.finalize()
    return nc


_cache = {}


def _get_program(key, cls_grid, causal):
    if key not in _cache:
        _cache[key] = _build(cls_grid, causal)
    return _cache[key]


def _classify(em_t):
    """em_t: exp(mask).T [S, S] (ks, qs). Returns tuple-of-tuples class grid
    [NKS][NCH]."""
    grid = []
    for i in range(NKS):
        row = []
        for j in range(NCH):
            t = em_t[i * 128:(i + 1) * 128, j * CH:(j + 1) * CH]
            mx = t.max()
            mn = t.min()
            if mx == 0.0:
                row.append(SKIP)
            elif mn == 1.0 and mx == 1.0:
                row.append(PLAIN)
            else:
                row.append(MASKED)
        grid.append(tuple(row))
    return tuple(grid)


def _causal_grid():
    g = []
    for i in range(NKS):
        row = []
        for j in range(NCH):
            if i >= 4 * j + 4:
                row.append(SKIP)
            elif i >= 4 * j:
                row.append(MASKED)
            else:
                row.append(PLAIN)
        g.append(tuple(row))
    return tuple(g)


def _is_exact_causal(emaskT_b):
    """True iff exp(mask).T's diagonal band is exactly the causal 0/1
    pattern (off-band is covered by the grid comparison)."""
    p = np.arange(128)[:, None]
    for jj in range(NCH):
        for i in range(4 * jj, 4 * jj + 4):
            t = emaskT_b[i * 128:(i + 1) * 128, jj * CH:(jj + 1) * CH]
            d = i - 4 * jj
            q = np.arange(CH)[None, :]
            want = (p - q + 128 * d <= 0).astype(np.float32)
            if not np.array_equal(t, want):
                return False
    return True


def kernel(hidden_states, cos, sin, attention_mask,
           q_w, k_w, v_w, q_b, k_b, v_b,
           q_A, q_B, k_A, k_B, v_A, v_B, o_w):
    f32 = np.float32
    hidden_states = np.asarray(hidden_states, dtype=f32)
    cos = np.asarray(cos, dtype=f32)
    sin = np.asarray(sin, dtype=f32)
    mask = np.asarray(attention_mask, dtype=f32)[:, 0]  # [B, S, S]

    # host-side shared prep
    with np.errstate(under="ignore", over="ignore"):
        emask = np.exp(np.minimum(mask, 80.0))  # [B, S, S]; clamp avoids inf
    emaskT = [np.ascontiguousarray(emask[b].T) for b in range(B)]
    grids = [_classify(emaskT[b]) for b in range(B)]
    if grids[0] != grids[1]:
        grid = tuple(tuple(MASKED if (grids[0][i][j] != SKIP or grids[1][i][j] != SKIP)
                           else SKIP for j in range(NCH)) for i in range(NKS))
    else:
        grid = grids[0]
    for j in range(NCH):
        if all(grid[i][j] == SKIP for i in range(NKS)):
            grid = tuple(tuple(MASKED for _ in range(NCH)) for _ in range(NKS))
            break

    causal = (grid == _causal_grid()
              and all(_is_exact_causal(emaskT[b]) for b in range(B)))

    nc = _get_program((grid, causal), grid, causal)

    # x_pre[c, p, kt, s'] = x[b][c*CH+s', kt*128+p]
    xT = [np.ascontiguousarray(
        hidden_states[b].reshape(NCH, CH, KT, 128).transpose(0, 3, 2, 1)
        ).astype(NPBF16) for b in range(B)]
    cosT = [np.ascontiguousarray(cos[b].T).astype(NPBF16) for b in range(B)]
    ss = np.concatenate([-sin[:, :, :HD // 2], sin[:, :, HD // 2:]], axis=-1)
    ssT = [np.ascontiguousarray(ss[b].T).astype(NPBF16) for b in range(B)]
    emaskT16 = None

    # effective weights: W_eff[outdim, h] = W[outdim, h] + s*(A @ B).T[outdim, h]
    qw_eff = q_w + LORA_SCALE * (q_A @ q_B).T
    kw_eff = k_w + LORA_SCALE * (k_A @ k_B).T
    vw_eff = v_w + LORA_SCALE * (v_A @ v_B).T

    in_maps = []
    for c in range(NCORES):
        b, g = divmod(c, KVH)
        qsl = slice(QD * g, QD * (g + 1))
        ksl = slice(HD * g, HD * (g + 1))
        w_cat = np.concatenate([qw_eff[qsl], kw_eff[ksl], vw_eff[ksl]], axis=0)
        # w_pre[p, t, kt, o] = w_cat[t*128+o, kt*128+p]
        wT_c = w_cat.reshape(NT, 128, KT, 128).transpose(3, 0, 2, 1)
        # v bias handled on host: after softmax-normalization its contribution
        # to the output is the constant row o_w @ vb_o (added post-gather)
        bias_cat = np.concatenate([q_b[qsl], k_b[ksl],
                                   np.zeros(HD, f32)]).astype(f32)
        bias_cols = bias_cat.reshape(NT, 128).T  # [128, NT]
        swap_idx = np.concatenate([np.arange(64, 128), np.arange(0, 64)])
        biasT_c = np.ascontiguousarray(
            np.concatenate([bias_cols, bias_cols[swap_idx]], axis=1))  # [128, 2*NT]
        owT_c = o_w[:, qsl].T
        m = {
            "xT": xT[b],
            "wT": np.ascontiguousarray(wT_c).astype(NPBF16),
            "biasT": biasT_c,
            "cachetag": np.zeros((1, (K_TAG_INT % 97) + 1), f32),
            "cosT": cosT[b],
            "ssT": ssT[b],
            "owT": np.ascontiguousarray(owT_c).astype(NPBF16),
        }
        if not causal and any(grid[i][j] == MASKED for i in range(NKS) for j in range(NCH)):
            if emaskT16 is None:
                emaskT16 = [e.astype(NPBF16) for e in emaskT]
            m["emaskT"] = emaskT16[b]
        in_maps.append(m)

    res = run_bass_kernel_spmd(nc, in_maps, core_ids=list(range(NCORES)))
    outs = [np.asarray(r["out_p"], dtype=f32) for r in res.results]
    # v-bias contribution: softmax rows sum to 1, so the +v_b term passes
    # through attention unchanged and adds o_w @ vb_o to every output row
    vb_o = np.empty(NH * HD, f32)
    for g in range(KVH):
        vb_o[QD * g:QD * (g + 1)] = np.tile(v_b[HD * g:HD * (g + 1)], GQ)
    delta = (o_w.astype(f32) @ vb_o)[None, :]  # [1, H]
    full = np.empty((B, S, H), f32)
    for b in range(B):
        full[b] = outs[KVH * b]
        for g in range(1, KVH):
            full[b] += outs[KVH * b + g]
        full[b] += delta
    return full


# revision 56
# speedup vs baseline: 1.0008x; 1.0008x over previous
"""Trainium2 Bass kernel for LoRA-fused QKV + RoPE + GQA causal attention + o_proj.

Problem (hardcoded): B=2, S=2048, H=2048, NH=16, KVH=4, HD=128, R=16.

Sharding: 8 cores = batch(2) x kv-head-group(4). Core c handles batch b=c//4,
kv head g=c%4 (q heads 4g..4g+3). Each core computes its 4 heads' attention and
a partial o_proj ([S,H] partial over its 512 o-dims); host sums 4 partials per
batch.

v2 design vs the fp32r baseline:
- All matmuls in bf16 (1 cycle/row like fp32r, but FWL weight loads and half
  the DMA/SBUF). PSUM accumulation stays fp32. LoRA is folded into W on the
  host (W_eff = W^T + scale*A@B), biases are applied by the scalar engine
  during PSUM evacuation (activation bias=AP per-partition column).
- Everything stays in "transposed space": projections produce qT/kT/vT [d, s],
  scoresT [ks, qs] feeds AV directly, o_proj consumes outT [d, s] stationary.
- Softmax: no max-subtraction; exp on the scalar engine over PAIRS of score
  tiles (one ACTIVATE per 2 PSUM banks halves the ~310ns/instr overhead);
  column sums via an all-ones stationary matmul into PSUM; normalization on
  DVE (reciprocal_approx_fast + gpsimd partition_broadcast + multiply).
- Causal mask applied multiplicatively as exp(mask) with SKIP tiles dropped;
  the diagonal-tile patterns are generated on device as paired bf16 tiles.
- Single fused loop over 4 s-chunks; o_proj of chunk j flushes at the start of
  chunk j+1; attention is software-pipelined 2 score-pairs deep across head
  boundaries so PE never waits on exp.
- Weights resident in SBUF (loaded once); x/cos/sin double-buffered and
  prefetched a chunk ahead; initial DMAs split across queues so the first
  matmul starts ~2us in; output tiles stream back round-robin on 4 queues.
"""

import hashlib
import numpy as np
import ml_dtypes

import concourse.bass as bass
import concourse.mybir as mybir
import concourse.tile as tile
from concourse import bacc
from concourse.bass_utils import run_bass_kernel_spmd

B, S, H = 2, 2048, 2048
NH, KVH, HD = 16, 4, 128
R = 16
LORA_SCALE = 32.0 / 16.0
ATTN_SCALE = HD ** -0.5

NCORES = 8
GQ = NH // KVH          # 4 q heads per core
NT = GQ + 2             # 6 projection tiles: 4 q heads, 1 k, 1 v
QD = GQ * HD            # 512
CH = 512                # s-chunk width
NCH = S // CH           # 4 s-chunks
KT = H // 128           # 16 contraction k-tiles
NKS = S // 128          # 16 ks tiles
F32 = mybir.dt.float32
F32R = mybir.dt.float32r
BF16 = mybir.dt.bfloat16
NPBF16 = ml_dtypes.bfloat16

# tile classification codes (host-computed from exp(mask) tiles)
SKIP, PLAIN, MASKED = 0, 1, 2

# content tag: force a fresh NEFF cache key whenever this file changes
with open(__file__, "rb") as _f:
    KTAG = hashlib.sha1(_f.read()).hexdigest()[:10]
K_TAG_INT = int(KTAG, 16)


def _build(cls_grid, causal):
    """Build the SPMD program. cls_grid[i][j] in {SKIP, PLAIN, MASKED} for
    scoresT tile (ks_tile i, qs_chunk j). causal=True generates the diagonal
    mask tiles on device (no emaskT input)."""
    nc = bacc.Bacc("TRN2", target_bir_lowering=False)

    # host-packed for contiguous per-partition DMA:
    # x_pre[c, p, kt, s'] = x[b][s = c*CH+s', h = kt*128+p]  (bf16)
    xT = nc.dram_tensor("xT", [NCH, 128, KT, CH], BF16, kind="ExternalInput")
    # w_pre[p, t, kt, o] = w_eff[h = kt*128+p, t*128+o]  (bf16, LoRA folded)
    wT = nc.dram_tensor("wT", [128, NT, KT, 128], BF16, kind="ExternalInput")
    # [:, 0:NT] plain bias columns; [:, NT:2*NT] partition-swapped (rotate-half)
    biasT = nc.dram_tensor("biasT", [128, 2 * NT], F32, kind="ExternalInput")
    # cache-buster: the PJRT NEFF cache hashes the HLO minus backend_config
    DL = (K_TAG_INT % 97) + 1
    dummy = nc.dram_tensor("cachetag", [1, DL], F32, kind="ExternalInput")
    cosT = nc.dram_tensor("cosT", [HD, S], BF16, kind="ExternalInput")
    ssT = nc.dram_tensor("ssT", [HD, S], BF16, kind="ExternalInput")
    any_masked = any(cls_grid[i][j] == MASKED for i in range(NKS) for j in range(NCH))
    emaskT = None
    if not causal and any_masked:
        emaskT = nc.dram_tensor("emaskT", [S, S], BF16, kind="ExternalInput")
    owT = nc.dram_tensor("owT", [QD, H], BF16, kind="ExternalInput")
    out_p = nc.dram_tensor("out_p", [S, H], BF16, kind="ExternalOutput")

    live_per_j = [[i for i in range(NKS) if cls_grid[i][jj] != SKIP]
                  for jj in range(NCH)]
    masked_per_j = [[i for i in range(NKS) if cls_grid[i][jj] == MASKED]
                    for jj in range(NCH)]
    need = [max(jj, max(live_per_j[jj]) // (CH // 128)) for jj in range(NCH)]
    QCH_BUFS = max(2, max(need[jj] - jj for jj in range(NCH)) + 1)

    with tile.TileContext(nc) as tc:
        from concourse.masks import make_identity
        with tc.tile_pool(name="consts", bufs=1) as consts, \
             tc.tile_pool(name="persist", bufs=1) as persist, \
             tc.tile_pool(name="qch", bufs=QCH_BUFS) as qch_pool, \
             tc.tile_pool(name="outp", bufs=2) as outp_pool, \
             tc.tile_pool(name="p1", bufs=5) as p1, \
             tc.tile_pool(name="xch", bufs=2) as xch_pool, \
             tc.tile_pool(name="att", bufs=3) as att_pool, \
             tc.tile_pool(name="stgp", bufs=8) as stgp, \
             tc.tile_pool(name="fin", bufs=2) as fin, \
             tc.tile_pool(name="pp_pair", bufs=2, space="PSUM") as pp_pair, \
             tc.tile_pool(name="pp_o", bufs=2, space="PSUM") as pp_o, \
             tc.tile_pool(name="pp_sum", bufs=1, space="PSUM") as pp_sum, \
             tc.tile_pool(name="pp_t", bufs=1, space="PSUM") as pp_t:

            # ---- chunk-0 inputs first: x pieces split across queues ----
            x_tiles = {}

            def emit_x_dma(c, fine=False):
                x_c = xch_pool.tile([128, KT, CH], BF16, tag="x_c", name=f"x_{c}")
                x_tiles[c] = x_c
                if fine:
                    # 16 single-kt pieces: first matmul starts after 0.13MB
                    for kt in range(KT):
                        q = nc.sync if kt % 2 == 0 else nc.gpsimd
                        q.dma_start(out=x_c[:, bass.ds(kt, 1), :],
                                    in_=xT[c, :, bass.ds(kt, 1), :])
                else:
                    qs = [nc.sync, nc.gpsimd, nc.sync, nc.gpsimd]
                    for kq in range(4):
                        qs[kq].dma_start(out=x_c[:, bass.ds(kq * 4, 4), :],
                                         in_=xT[c, :, bass.ds(kq * 4, 4), :])

            cs_tiles = {}

            def emit_cs_dma(c):
                sl = bass.ds(c * CH, CH)
                cos_c = xch_pool.tile([128, CH], BF16, tag="cos_c", name=f"cos_{c}")
                nc.sync.dma_start(out=cos_c, in_=cosT[:, sl])
                ss_c = xch_pool.tile([128, CH], BF16, tag="ss_c", name=f"ss_{c}")
                nc.sync.dma_start(out=ss_c, in_=ssT[:, sl])
                cs_tiles[c] = (cos_c, ss_c)

            # ---- weights resident, all on the scalar queue in t-need order;
            # x chunk 0 finely split on sync/gpsimd in parallel ----
            w_sb = persist.tile([128, NT, KT, 128], BF16, tag="w_sb")
            # t0 in 4 fine pieces so the first matmul starts after ~0.16MB
            for kq in range(4):
                nc.scalar.dma_start(out=w_sb[:, 0, bass.ds(kq * 4, 4), :],
                                    in_=wT[:, 0, bass.ds(kq * 4, 4), :])
            emit_x_dma(0, fine=True)
            for t in [1, GQ, 2, 3, NT - 1]:   # proj t-need order
                nc.scalar.dma_start(out=w_sb[:, t, :, :], in_=wT[:, t, :, :])
            emit_cs_dma(0)
            bias_sb = consts.tile([128, 2 * NT], F32, tag="bias_sb")
            nc.gpsimd.dma_start(out=bias_sb, in_=biasT[:, :])
            dummy_sb = consts.tile([1, 128], F32, tag="dummy_sb")
            nc.gpsimd.dma_start(out=dummy_sb[:, 0:DL], in_=dummy[:, :])

            # ---- small constants ----
            # full 128-col all-ones stationary: the denominator matmul then
            # broadcasts the column sums across all 128 PSUM partitions (no
            # gpsimd partition_broadcast needed) and keeps LDWEIGHTS
            # pull-ahead working (no col_grp restriction)
            ones_mat = consts.tile([128, 128], BF16, tag="ones_mat")
            nc.vector.memset(ones_mat, 1.0)
            ident_f = consts.tile([128, 128], F32, tag="ident_f")
            make_identity(nc, ident_f)
            identb = consts.tile([128, 128], BF16, tag="identb")
            nc.vector.tensor_copy(out=identb, in_=ident_f)

            # half-rotation permutation: perm[p, q] = 1 iff q == (p+64)%128.
            # Used as a matmul stationary to compute rotate-half on the PE
            # (swap via DMA would cross partitions on a busy queue).
            perm_f = consts.tile([128, 128], F32, tag="perm_f")
            nc.gpsimd.memset(perm_f, 0.0)
            for base in (64, -64):
                nc.gpsimd.affine_select(
                    out=perm_f, in_=perm_f,
                    compare_op=mybir.AluOpType.not_equal,
                    fill=1.0,
                    base=base,
                    channel_multiplier=1,
                    pattern=[[-1, 128]],
                )
            permb = consts.tile([128, 128], BF16, tag="permb")
            nc.vector.tensor_copy(out=permb, in_=perm_f)

            # causal diagonal mask tiles, paired: pattern depends on d = i - 4j
            # m_d[p, q] = 1.0 if p - q + 128*d <= 0 else 0.0
            diag_pairs = []
            if causal:
                scratch = consts.tile([128, CH], F32, tag="diag_scratch")
                for pp in range(2):
                    mp = consts.tile([128, 2, CH], BF16, tag=f"diagp{pp}",
                                     name=f"diagp{pp}")
                    for half in range(2):
                        dd = 2 * pp + half
                        nc.gpsimd.memset(scratch, 0.0)
                        nc.gpsimd.affine_select(
                            out=scratch, in_=scratch,
                            compare_op=mybir.AluOpType.is_gt,
                            fill=1.0,
                            base=128 * dd,
                            channel_multiplier=1,
                            pattern=[[-1, CH]],
                        )
                        nc.vector.tensor_copy(out=mp[:, half, :], in_=scratch)
                    diag_pairs.append(mp)

            # ow and next-chunk inputs are prefetched lazily (first attention)
            # so they don't steal startup fabric bandwidth from x(0)/w
            ow_sb = persist.tile([128, GQ, H], BF16, tag="ow_sb")
            ow_done = [False]

            def emit_prefetch(c):
                if c + 1 < NCH:
                    emit_x_dma(c + 1)
                    emit_cs_dma(c + 1)
                if not ow_done[0]:
                    ow_done[0] = True
                    nc.gpsimd.dma_start(
                        out=ow_sb, in_=owT.rearrange("(g p) n -> p g n", p=128))

            # ---- persistent tiles ----
            kT_full = persist.tile([128, S], BF16, tag="kT_full")
            v_nat = persist.tile([128, NKS, 128], BF16, tag="v_nat")  # [ks, tile, d]

            out_dma_q = [nc.sync, nc.gpsimd]
            out_dma_n = [0]

            def emit_oproj(args, final=False):
                cc, outT_ch = args
                qlist = [nc.sync, nc.gpsimd, nc.scalar] if final else [nc.gpsimd, nc.scalar]
                for st4 in range(CH // 128):
                    ssl = bass.ds(st4 * 128, 128)
                    dsl = bass.ds((cc * (CH // 128) + st4) * 128, 128)
                    for nch in range(NCH):
                        pop_deferred_v()
                        nsl = bass.ds(nch * CH, CH)
                        ps3 = pp_pair.tile([128, 2, CH], F32, tag="pair", name="ps3")
                        g = st4 * NCH + nch
                        half = g % 2
                        for h in range(GQ):
                            nc.tensor.matmul(ps3[:, half, :], outT_ch[h][:, ssl],
                                             ow_sb[:, h, nsl],
                                             start=(h == 0), stop=(h == GQ - 1))
                        stg = stgp.tile([128, CH], BF16, tag="stg")
                        if g % 2 == 0:
                            nc.vector.tensor_copy(out=stg, in_=ps3[:, half, :])
                        else:
                            nc.scalar.activation(out=stg, in_=ps3[:, half, :],
                                                 func=mybir.ActivationFunctionType.Copy)
                        q = qlist[out_dma_n[0] % len(qlist)]
                        out_dma_n[0] += 1
                        q.dma_start(out=out_p[dsl, nsl], in_=stg)

            q_chunks = {}
            deferred_v = []

            def pop_deferred_v():
                """Emit ONE pending v transpose (or nothing)."""
                if not deferred_v:
                    return
                c, vT_c, i4 = deferred_v.pop(0)
                i = c * (CH // 128) + i4
                ps_t = pp_t.tile([128, 128], BF16, tag="ps_t", name="ps_t")
                nc.tensor.transpose(ps_t, vT_c[:, bass.ds(i4 * 128, 128)], identb)
                nc.vector.tensor_copy(out=v_nat[:, i, :], in_=ps_t)

            def flush_deferred_v():
                while deferred_v:
                    pop_deferred_v()

            def emit_proj(c):
                x_c = x_tiles[c]
                cos_c, ss_c = cs_tiles[c]
                sl = bass.ds(c * CH, CH)

                q_ch = [qch_pool.tile([128, CH], BF16, tag=f"qch{h}", name=f"qch{h}_{c}")
                        for h in range(GQ)]
                q_chunks[c] = q_ch

                pending_rope = []

                def flush_rope():
                    # rotate-half via PE permutation matmul, then rope on DVE:
                    # dst = (raw+b)*cos + (perm@raw + swap(b))*ss
                    for t, raw in pending_rope:
                        bias_col = bias_sb[:, t:t + 1]
                        bias_sw = bias_sb[:, NT + t:NT + t + 1]
                        ps_sw = pp_t.tile([128, CH], F32, tag="ps_t", name="ps_sw")
                        nc.tensor.matmul(ps_sw, permb, raw, start=True, stop=True)
                        sw = p1.tile([128, CH], BF16, tag="sw")
                        nc.vector.scalar_tensor_tensor(
                            out=sw, in0=ps_sw, scalar=bias_sw, in1=ss_c,
                            op0=mybir.AluOpType.add, op1=mybir.AluOpType.mult)
                        dst = q_ch[t] if t < GQ else kT_full[:, sl]
                        nc.vector.scalar_tensor_tensor(
                            out=dst, in0=raw, scalar=bias_col, in1=cos_c,
                            op0=mybir.AluOpType.add, op1=mybir.AluOpType.mult)
                        nc.vector.tensor_add(dst, dst, sw)
                    pending_rope.clear()

                # t order: q0, q1, k, q2, q3, v; each tile's rope flushes one
                # MM-group later so the PSUM evacuation copy is never waited on
                t_list = [0, 1, GQ, 2, 3, NT - 1]
                for n, t in enumerate(t_list):
                    ps = pp_pair.tile([128, 2, CH], F32, tag="pair", name="ps_p")
                    half = n % 2
                    for kt in range(KT):
                        nc.tensor.matmul(ps[:, half, :], w_sb[:, t, kt, :],
                                         x_c[:, kt, :],
                                         start=(kt == 0), stop=(kt == KT - 1))
                    if t == NT - 1:   # v (bias folded on host); transpose deferred
                        vT_c = p1.tile([128, CH], BF16, tag="vT_c")
                        nc.scalar.activation(out=vT_c, in_=ps[:, half, :],
                                             func=mybir.ActivationFunctionType.Copy)
                        for i4 in range(CH // 128):
                            deferred_v.append((c, vT_c, i4))
                    else:
                        raw = p1.tile([128, CH], BF16, tag="raw")
                        nc.scalar.activation(out=raw, in_=ps[:, half, :],
                                             func=mybir.ActivationFunctionType.Copy)
                        flush_rope()
                        pending_rope.append((t, raw))
                flush_rope()

            def emit_attention(j):
                """Emits the attention for chunk j with a 2-pair software
                pipeline across head boundaries."""
                flush_deferred_v()
                sl = bass.ds(j * CH, CH)
                live = live_per_j[j]
                masked = set(masked_per_j[j])
                # build pair list: (i0, i1 or None)
                pairs = []
                for n in range(0, len(live), 2):
                    i0 = live[n]
                    i1 = live[n + 1] if n + 1 < len(live) else None
                    pairs.append((i0, i1))

                outT_ch = [outp_pool.tile([128, CH], BF16, tag=f"outT{h}",
                                          name=f"outT{h}_{j}") for h in range(GQ)]
                q_ch = q_chunks[j]

                # global pipeline over (head, pair)
                work = [(h, p) for h in range(GQ) for p in pairs]
                NP = len(work)
                attns = {}

                def emit_qk_exp(n):
                    h, (i0, i1) = work[n]
                    qh = q_ch[h]
                    pr = pp_pair.tile([128, 2, CH], F32, tag="pair", name="pr")
                    nc.tensor.matmul(pr[:, 0, :], kT_full[:, bass.ds(i0 * 128, 128)],
                                     qh, start=True, stop=True)
                    if i1 is not None:
                        nc.tensor.matmul(pr[:, 1, :],
                                         kT_full[:, bass.ds(i1 * 128, 128)],
                                         qh, start=True, stop=True)
                    attn = att_pool.tile([128, 2, CH], BF16, tag="attn")
                    src = pr if i1 is not None else pr[:, 0:1, :]
                    dst = attn if i1 is not None else attn[:, 0:1, :]
                    nc.scalar.activation(out=dst, in_=src,
                                         func=mybir.ActivationFunctionType.Exp,
                                         scale=float(ATTN_SCALE))
                    pair_masked = (i0 in masked) or (i1 in masked)
                    if pair_masked:
                        if causal:
                            # diag tiles are i = 4j + d; pairs aligned (d0,d1),(d2,d3)
                            dd = i0 - 4 * j
                            nc.vector.tensor_mul(attn, attn, diag_pairs[dd // 2])
                        else:
                            mt = att_pool.tile([128, 2, CH], BF16, tag="m_tile",
                                               bufs=3, name=f"mt_{j}_{h}_{i0}")
                            nc.gpsimd.dma_start(out=mt[:, 0, :],
                                                in_=emaskT[bass.ds(i0 * 128, 128), sl])
                            if i1 is not None:
                                nc.gpsimd.dma_start(
                                    out=mt[:, 1, :],
                                    in_=emaskT[bass.ds(i1 * 128, 128), sl])
                            msl = attn if i1 is not None else attn[:, 0:1, :]
                            mm = mt if i1 is not None else mt[:, 0:1, :]
                            nc.vector.tensor_mul(msl, msl, mm)
                    attns[n] = attn

                def emit_av(n):
                    h, (i0, i1) = work[n]
                    attn = attns.pop(n)
                    pidx = n % len(pairs)
                    first, last = (pidx == 0), (pidx == len(pairs) - 1)
                    ps_o = st_o[h]
                    ps_sum = st_sum[h]
                    nc.tensor.matmul(ps_o, v_nat[:, i0, :], attn[:, 0, :],
                                     start=first, stop=(last and i1 is None))
                    nc.tensor.matmul(ps_sum, ones_mat, attn[:, 0, :],
                                     start=first, stop=(last and i1 is None))
                    if i1 is not None:
                        nc.tensor.matmul(ps_o, v_nat[:, i1, :], attn[:, 1, :],
                                         start=False, stop=last)
                        nc.tensor.matmul(ps_sum, ones_mat, attn[:, 1, :],
                                         start=False, stop=last)
                    if last:
                        finalize(h)

                st_o, st_sum = {}, {}

                def start_head(h):
                    st_o[h] = pp_o.tile([128, CH], F32, tag="ps_o", name="ps_o")
                    st_sum[h] = pp_sum.tile([128, CH], F32, tag="ps_sum",
                                            name="ps_sum")

                def finalize(h):
                    # ps_sum already holds the denominator in every partition
                    recip = fin.tile([128, CH], F32, tag="recip")
                    nc.vector.reciprocal_approx_fast(out=recip, in_=st_sum[h])
                    nc.vector.tensor_mul(outT_ch[h], st_o[h], recip)

                PIPE = 2
                npairs = len(pairs)
                for n in range(NP):
                    if n % npairs == 0:
                        start_head(work[n][0])
                    emit_qk_exp(n)
                    if n >= PIPE:
                        emit_av(n - PIPE)
                for n in range(max(0, NP - PIPE), NP):
                    emit_av(n)
                return outT_ch

            # ---- main fused loop: proj(c) -> o_proj(c-1) flush (outT finalize
            # gets proj-length slack) -> x(c+1) prefetch -> attention(c) ----
            pending_oproj = None
            for c in range(NCH):
                emit_proj(c)
                if pending_oproj is not None:
                    emit_oproj(pending_oproj)
                    pending_oproj = None
                first_att = True
                for j in range(NCH):
                    if need[j] == c:
                        if pending_oproj is not None:
                            emit_oproj(pending_oproj)
                            pending_oproj = None
                        if first_att:
                            # prefetch next chunk's x/cos/sin during attention
                            emit_prefetch(c)
                            first_att = False
                        pending_oproj = (j, emit_attention(j))
                if first_att:
                    emit_prefetch(c)

            if pending_oproj is not None:
                emit_oproj(pending_oproj, final=True)
                pending_oproj = None

    nc.finalize()
    return nc


_cache = {}


def _get_program(key, cls_grid, causal):
    if key not in _cache:
        _cache[key] = _build(cls_grid, causal)
    return _cache[key]


def _classify(em_t):
    """em_t: exp(mask).T [S, S] (ks, qs). Returns tuple-of-tuples class grid
    [NKS][NCH]."""
    grid = []
    for i in range(NKS):
        row = []
        for j in range(NCH):
            t = em_t[i * 128:(i + 1) * 128, j * CH:(j + 1) * CH]
            mx = t.max()
            mn = t.min()
            if mx == 0.0:
                row.append(SKIP)
            elif mn == 1.0 and mx == 1.0:
                row.append(PLAIN)
            else:
                row.append(MASKED)
        grid.append(tuple(row))
    return tuple(grid)


def _causal_grid():
    g = []
    for i in range(NKS):
        row = []
        for j in range(NCH):
            if i >= 4 * j + 4:
                row.append(SKIP)
            elif i >= 4 * j:
                row.append(MASKED)
            else:
                row.append(PLAIN)
        g.append(tuple(row))
    return tuple(g)


def _is_exact_causal(emaskT_b):
    """True iff exp(mask).T's diagonal band is exactly the causal 0/1
    pattern (off-band is covered by the grid comparison)."""
    p = np.arange(128)[:, None]
    for jj in range(NCH):
        for i in range(4 * jj, 4 * jj + 4):
            t = emaskT_b[i * 128:(i + 1) * 128, jj * CH:(jj + 1) * CH]
            d = i - 4 * jj
            q = np.arange(CH)[None, :]
            want = (p - q + 128 * d <= 0).astype(np.float32)
            if not np.array_equal(t, want):
                return False
    return True


def kernel(hidden_states, cos, sin, attention_mask,
           q_w, k_w, v_w, q_b, k_b, v_b,
           q_A, q_B, k_A, k_B, v_A, v_B, o_w):
    f32 = np.float32
    hidden_states = np.asarray(hidden_states, dtype=f32)
    cos = np.asarray(cos, dtype=f32)
    sin = np.asarray(sin, dtype=f32)
    mask = np.asarray(attention_mask, dtype=f32)[:, 0]  # [B, S, S]

    # host-side shared prep
    with np.errstate(under="ignore", over="ignore"):
        emask = np.exp(np.minimum(mask, 80.0))  # [B, S, S]; clamp avoids inf
    emaskT = [np.ascontiguousarray(emask[b].T) for b in range(B)]
    grids = [_classify(emaskT[b]) for b in range(B)]
    if grids[0] != grids[1]:
        grid = tuple(tuple(MASKED if (grids[0][i][j] != SKIP or grids[1][i][j] != SKIP)
                           else SKIP for j in range(NCH)) for i in range(NKS))
    else:
        grid = grids[0]
    for j in range(NCH):
        if all(grid[i][j] == SKIP for i in range(NKS)):
            grid = tuple(tuple(MASKED for _ in range(NCH)) for _ in range(NKS))
            break

    causal = (grid == _causal_grid()
              and all(_is_exact_causal(emaskT[b]) for b in range(B)))

    nc = _get_program((grid, causal), grid, causal)

    # x_pre[c, p, kt, s'] = x[b][c*CH+s', kt*128+p]
    xT = [np.ascontiguousarray(
        hidden_states[b].reshape(NCH, CH, KT, 128).transpose(0, 3, 2, 1)
        ).astype(NPBF16) for b in range(B)]
    cosT = [np.ascontiguousarray(cos[b].T).astype(NPBF16) for b in range(B)]
    ss = np.concatenate([-sin[:, :, :HD // 2], sin[:, :, HD // 2:]], axis=-1)
    ssT = [np.ascontiguousarray(ss[b].T).astype(NPBF16) for b in range(B)]
    emaskT16 = None

    # effective weights: W_eff[outdim, h] = W[outdim, h] + s*(A @ B).T[outdim, h]
    qw_eff = q_w + LORA_SCALE * (q_A @ q_B).T
    kw_eff = k_w + LORA_SCALE * (k_A @ k_B).T
    vw_eff = v_w + LORA_SCALE * (v_A @ v_B).T

    in_maps = []
    for c in range(NCORES):
        b, g = divmod(c, KVH)
        qsl = slice(QD * g, QD * (g + 1))
        ksl = slice(HD * g, HD * (g + 1))
        w_cat = np.concatenate([qw_eff[qsl], kw_eff[ksl], vw_eff[ksl]], axis=0)
        # w_pre[p, t, kt, o] = w_cat[t*128+o, kt*128+p]
        wT_c = w_cat.reshape(NT, 128, KT, 128).transpose(3, 0, 2, 1)
        # v bias handled on host: after softmax-normalization its contribution
        # to the output is the constant row o_w @ vb_o (added post-gather)
        bias_cat = np.concatenate([q_b[qsl], k_b[ksl],
                                   np.zeros(HD, f32)]).astype(f32)
        bias_cols = bias_cat.reshape(NT, 128).T  # [128, NT]
        swap_idx = np.concatenate([np.arange(64, 128), np.arange(0, 64)])
        biasT_c = np.ascontiguousarray(
            np.concatenate([bias_cols, bias_cols[swap_idx]], axis=1))  # [128, 2*NT]
        owT_c = o_w[:, qsl].T
        m = {
            "xT": xT[b],
            "wT": np.ascontiguousarray(wT_c).astype(NPBF16),
            "biasT": biasT_c,
            "cachetag": np.zeros((1, (K_TAG_INT % 97) + 1), f32),
            "cosT": cosT[b],
            "ssT": ssT[b],
            "owT": np.ascontiguousarray(owT_c).astype(NPBF16),
        }
        if not causal and any(grid[i][j] == MASKED for i in range(NKS) for j in range(NCH)):
            if emaskT16 is None:
                emaskT16 = [e.astype(NPBF16) for e in emaskT]
            m["emaskT"] = emaskT16[b]
        in_maps.append(m)

    res = run_bass_kernel_spmd(nc, in_maps, core_ids=list(range(NCORES)))
    outs = [np.asarray(r["out_p"], dtype=f32) for r in res.results]
    # v-bias contribution: softmax rows sum to 1, so the +v_b term passes
    # through attention unchanged and adds o_w @ vb_o to every output row
    vb_o = np.empty(NH * HD, f32)
    for g in range(KVH):
        vb_o[QD * g:QD * (g + 1)] = np.tile(v_b[HD * g:HD * (g + 1)], GQ)
    delta = (o_w.astype(f32) @ vb_o)[None, :]  # [1, H]
    full = np.empty((B, S, H), f32)
    for b in range(B):
        full[b] = outs[KVH * b]
        for g in range(1, KVH):
            full[b] += outs[KVH * b + g]
        full[b] += delta
    return full


# revision 59
# speedup vs baseline: 1.1892x; 1.1883x over previous
"""Trainium2 Bass kernel for LoRA-fused QKV + RoPE + GQA causal attention + o_proj.

Problem (hardcoded): B=2, S=2048, H=2048, NH=16, KVH=4, HD=128, R=16.

Sharding: 8 cores = batch(2) x kv-head-group(4). Core c handles batch b=c//4,
kv head g=c%4 (q heads 4g..4g+3). Each core computes its 4 heads' attention and
a partial o_proj ([S,H] partial over its 512 o-dims); host sums 4 partials per
batch.

v2 design vs the fp32r baseline:
- All matmuls in bf16 (1 cycle/row like fp32r, but FWL weight loads and half
  the DMA/SBUF). PSUM accumulation stays fp32. LoRA is folded into W on the
  host (W_eff = W^T + scale*A@B), biases are applied by the scalar engine
  during PSUM evacuation (activation bias=AP per-partition column).
- Everything stays in "transposed space": projections produce qT/kT/vT [d, s],
  scoresT [ks, qs] feeds AV directly, o_proj consumes outT [d, s] stationary.
- Softmax: no max-subtraction; exp on the scalar engine over PAIRS of score
  tiles (one ACTIVATE per 2 PSUM banks halves the ~310ns/instr overhead);
  column sums via an all-ones stationary matmul into PSUM; normalization on
  DVE (reciprocal_approx_fast + gpsimd partition_broadcast + multiply).
- Causal mask applied multiplicatively as exp(mask) with SKIP tiles dropped;
  the diagonal-tile patterns are generated on device as paired bf16 tiles.
- Single fused loop over 4 s-chunks; o_proj of chunk j flushes at the start of
  chunk j+1; attention is software-pipelined 2 score-pairs deep across head
  boundaries so PE never waits on exp.
- Weights resident in SBUF (loaded once); x/cos/sin double-buffered and
  prefetched a chunk ahead; initial DMAs split across queues so the first
  matmul starts ~2us in; output tiles stream back round-robin on 4 queues.
"""

import hashlib
import numpy as np
import ml_dtypes

import concourse.bass as bass
import concourse.mybir as mybir
import concourse.tile as tile
from concourse import bacc
from concourse.bass_utils import run_bass_kernel_spmd

B, S, H = 2, 2048, 2048
NH, KVH, HD = 16, 4, 128
R = 16
LORA_SCALE = 32.0 / 16.0
ATTN_SCALE = HD ** -0.5

NCORES = 8
GQ = NH // KVH          # 4 q heads per core
NT = GQ + 2             # 6 projection tiles: 4 q heads, 1 k, 1 v
QD = GQ * HD            # 512
CH = 512                # s-chunk width
NCH = S // CH           # 4 s-chunks
KT = H // 128           # 16 contraction k-tiles
NKS = S // 128          # 16 ks tiles
F32 = mybir.dt.float32
F32R = mybir.dt.float32r
BF16 = mybir.dt.bfloat16
NPBF16 = ml_dtypes.bfloat16

# tile classification codes (host-computed from exp(mask) tiles)
SKIP, PLAIN, MASKED = 0, 1, 2

# content tag: force a fresh NEFF cache key whenever this file changes
with open(__file__, "rb") as _f:
    KTAG = hashlib.sha1(_f.read()).hexdigest()[:10]
K_TAG_INT = int(KTAG, 16)


def _build(cls_grid, causal):
    """Build the SPMD program. cls_grid[i][j] in {SKIP, PLAIN, MASKED} for
    scoresT tile (ks_tile i, qs_chunk j). causal=True generates the diagonal
    mask tiles on device (no emaskT input)."""
    nc = bacc.Bacc("TRN2", target_bir_lowering=False)

    # host-packed for contiguous per-partition DMA:
    # x_pre[c, p, kt, s'] = x[b][s = c*CH+s', h = kt*128+p]  (bf16)
    xT = nc.dram_tensor("xT", [NCH, 128, KT, CH], BF16, kind="ExternalInput")
    # w_pre[p, t, kt, o] = w_eff[h = kt*128+p, t*128+o]  (bf16, LoRA folded)
    wT = nc.dram_tensor("wT", [128, NT, KT, 128], BF16, kind="ExternalInput")
    # [:, 0:NT] plain bias columns; [:, NT:2*NT] partition-swapped (rotate-half)
    biasT = nc.dram_tensor("biasT", [128, 2 * NT], F32, kind="ExternalInput")
    # cache-buster: the PJRT NEFF cache hashes the HLO minus backend_config
    DL = (K_TAG_INT % 97) + 1
    dummy = nc.dram_tensor("cachetag", [1, DL], F32, kind="ExternalInput")
    cosT = nc.dram_tensor("cosT", [HD, S], BF16, kind="ExternalInput")
    ssT = nc.dram_tensor("ssT", [HD, S], BF16, kind="ExternalInput")
    any_masked = any(cls_grid[i][j] == MASKED for i in range(NKS) for j in range(NCH))
    emaskT = None
    if not causal and any_masked:
        emaskT = nc.dram_tensor("emaskT", [S, S], BF16, kind="ExternalInput")
    owT = nc.dram_tensor("owT", [QD, H], BF16, kind="ExternalInput")
    out_p = nc.dram_tensor("out_p", [S, H], BF16, kind="ExternalOutput")

    live_per_j = [[i for i in range(NKS) if cls_grid[i][jj] != SKIP]
                  for jj in range(NCH)]
    masked_per_j = [[i for i in range(NKS) if cls_grid[i][jj] == MASKED]
                    for jj in range(NCH)]
    need = [max(jj, max(live_per_j[jj]) // (CH // 128)) for jj in range(NCH)]
    QCH_BUFS = max(2, max(need[jj] - jj for jj in range(NCH)) + 1)

    with tile.TileContext(nc) as tc:
        from concourse.masks import make_identity
        with tc.tile_pool(name="consts", bufs=1) as consts, \
             tc.tile_pool(name="persist", bufs=1) as persist, \
             tc.tile_pool(name="qch", bufs=QCH_BUFS) as qch_pool, \
             tc.tile_pool(name="outp", bufs=2) as outp_pool, \
             tc.tile_pool(name="p1", bufs=5) as p1, \
             tc.tile_pool(name="xch", bufs=2) as xch_pool, \
             tc.tile_pool(name="att", bufs=3) as att_pool, \
             tc.tile_pool(name="stgp", bufs=8) as stgp, \
             tc.tile_pool(name="fin", bufs=2) as fin, \
             tc.tile_pool(name="pp_pair", bufs=2, space="PSUM") as pp_pair, \
             tc.tile_pool(name="pp_o", bufs=2, space="PSUM") as pp_o, \
             tc.tile_pool(name="pp_sum", bufs=1, space="PSUM") as pp_sum, \
             tc.tile_pool(name="pp_t", bufs=1, space="PSUM") as pp_t:

            # ---- chunk-0 inputs first: x pieces split across queues ----
            x_tiles = {}

            def emit_x_dma(c, fine=False):
                x_c = xch_pool.tile([128, KT, CH], BF16, tag="x_c", name=f"x_{c}")
                x_tiles[c] = x_c
                if fine:
                    # 16 single-kt pieces: first matmul starts after 0.13MB
                    for kt in range(KT):
                        q = nc.sync if kt % 2 == 0 else nc.gpsimd
                        q.dma_start(out=x_c[:, bass.ds(kt, 1), :],
                                    in_=xT[c, :, bass.ds(kt, 1), :])
                else:
                    qs = [nc.sync, nc.gpsimd, nc.sync, nc.gpsimd]
                    for kq in range(4):
                        qs[kq].dma_start(out=x_c[:, bass.ds(kq * 4, 4), :],
                                         in_=xT[c, :, bass.ds(kq * 4, 4), :])

            cs_tiles = {}

            def emit_cs_dma(c):
                sl = bass.ds(c * CH, CH)
                cos_c = xch_pool.tile([128, CH], BF16, tag="cos_c", name=f"cos_{c}")
                nc.sync.dma_start(out=cos_c, in_=cosT[:, sl])
                ss_c = xch_pool.tile([128, CH], BF16, tag="ss_c", name=f"ss_{c}")
                nc.sync.dma_start(out=ss_c, in_=ssT[:, sl])
                cs_tiles[c] = (cos_c, ss_c)

            # ---- weights resident, all on the scalar queue in t-need order;
            # x chunk 0 finely split on sync/gpsimd in parallel ----
            w_sb = persist.tile([128, NT, KT, 128], BF16, tag="w_sb")
            # t0 in 4 fine pieces so the first matmul starts after ~0.16MB
            for kq in range(4):
                nc.scalar.dma_start(out=w_sb[:, 0, bass.ds(kq * 4, 4), :],
                                    in_=wT[:, 0, bass.ds(kq * 4, 4), :])
            emit_x_dma(0, fine=True)
            for t in [1, GQ, 2, 3, NT - 1]:   # proj t-need order
                nc.scalar.dma_start(out=w_sb[:, t, :, :], in_=wT[:, t, :, :])
            emit_cs_dma(0)
            bias_sb = consts.tile([128, 2 * NT], F32, tag="bias_sb")
            nc.gpsimd.dma_start(out=bias_sb, in_=biasT[:, :])
            dummy_sb = consts.tile([1, 128], F32, tag="dummy_sb")
            nc.gpsimd.dma_start(out=dummy_sb[:, 0:DL], in_=dummy[:, :])

            # ---- small constants ----
            # full 128-col all-ones stationary: the denominator matmul then
            # broadcasts the column sums across all 128 PSUM partitions (no
            # gpsimd partition_broadcast needed) and keeps LDWEIGHTS
            # pull-ahead working (no col_grp restriction)
            ones_mat = consts.tile([128, 128], BF16, tag="ones_mat")
            nc.vector.memset(ones_mat, 1.0)
            ident_f = consts.tile([128, 128], F32, tag="ident_f")
            make_identity(nc, ident_f)
            identb = consts.tile([128, 128], BF16, tag="identb")
            nc.vector.tensor_copy(out=identb, in_=ident_f)

            # half-rotation permutation: perm[p, q] = 1 iff q == (p+64)%128.
            # Used as a matmul stationary to compute rotate-half on the PE
            # (swap via DMA would cross partitions on a busy queue).
            perm_f = consts.tile([128, 128], F32, tag="perm_f")
            nc.gpsimd.memset(perm_f, 0.0)
            for base in (64, -64):
                nc.gpsimd.affine_select(
                    out=perm_f, in_=perm_f,
                    compare_op=mybir.AluOpType.not_equal,
                    fill=1.0,
                    base=base,
                    channel_multiplier=1,
                    pattern=[[-1, 128]],
                )
            permb = consts.tile([128, 128], BF16, tag="permb")
            nc.vector.tensor_copy(out=permb, in_=perm_f)

            # causal: single [128,128] lower-triangle mask; diag tiles are
            # processed as singles restricted to their live column span
            # [128*d, CH), where only the first 128 columns are triangular
            tri = None
            if causal:
                scratch = consts.tile([128, 128], F32, tag="tri_scratch")
                nc.gpsimd.memset(scratch, 0.0)
                nc.gpsimd.affine_select(
                    out=scratch, in_=scratch,
                    compare_op=mybir.AluOpType.is_gt,
                    fill=1.0,
                    base=0,
                    channel_multiplier=1,
                    pattern=[[-1, 128]],
                )
                tri = consts.tile([128, 128], BF16, tag="tri")
                nc.vector.tensor_copy(out=tri, in_=scratch)

            # ow and next-chunk inputs are prefetched lazily (first attention)
            # so they don't steal startup fabric bandwidth from x(0)/w
            ow_sb = persist.tile([128, GQ, H], BF16, tag="ow_sb")
            ow_done = [False]

            def emit_prefetch(c):
                if c + 1 < NCH:
                    emit_x_dma(c + 1)
                    emit_cs_dma(c + 1)
                if not ow_done[0]:
                    ow_done[0] = True
                    nc.gpsimd.dma_start(
                        out=ow_sb, in_=owT.rearrange("(g p) n -> p g n", p=128))

            # ---- persistent tiles ----
            kT_full = persist.tile([128, S], BF16, tag="kT_full")
            v_nat = persist.tile([128, NKS, 128], BF16, tag="v_nat")  # [ks, tile, d]

            out_dma_q = [nc.sync, nc.gpsimd]
            out_dma_n = [0]

            def emit_oproj(args, final=False):
                cc, outT_ch = args
                qlist = [nc.sync, nc.gpsimd, nc.scalar] if final else [nc.gpsimd, nc.scalar]
                for st4 in range(CH // 128):
                    ssl = bass.ds(st4 * 128, 128)
                    dsl = bass.ds((cc * (CH // 128) + st4) * 128, 128)
                    for nch in range(NCH):
                        pop_deferred_v()
                        nsl = bass.ds(nch * CH, CH)
                        ps3 = pp_pair.tile([128, 2, CH], F32, tag="pair", name="ps3")
                        g = st4 * NCH + nch
                        half = g % 2
                        for h in range(GQ):
                            nc.tensor.matmul(ps3[:, half, :], outT_ch[h][:, ssl],
                                             ow_sb[:, h, nsl],
                                             start=(h == 0), stop=(h == GQ - 1))
                        stg = stgp.tile([128, CH], BF16, tag="stg")
                        if g % 2 == 0:
                            nc.vector.tensor_copy(out=stg, in_=ps3[:, half, :])
                        else:
                            nc.scalar.activation(out=stg, in_=ps3[:, half, :],
                                                 func=mybir.ActivationFunctionType.Copy)
                        q = qlist[out_dma_n[0] % len(qlist)]
                        out_dma_n[0] += 1
                        q.dma_start(out=out_p[dsl, nsl], in_=stg)

            q_chunks = {}
            deferred_v = []

            def pop_deferred_v():
                """Emit ONE pending v transpose (or nothing)."""
                if not deferred_v:
                    return
                c, vT_c, i4 = deferred_v.pop(0)
                i = c * (CH // 128) + i4
                ps_t = pp_t.tile([128, 128], BF16, tag="ps_t", name="ps_t")
                nc.tensor.transpose(ps_t, vT_c[:, bass.ds(i4 * 128, 128)], identb)
                nc.vector.tensor_copy(out=v_nat[:, i, :], in_=ps_t)

            def flush_deferred_v():
                while deferred_v:
                    pop_deferred_v()

            def emit_proj(c):
                x_c = x_tiles[c]
                cos_c, ss_c = cs_tiles[c]
                sl = bass.ds(c * CH, CH)

                q_ch = [qch_pool.tile([128, CH], BF16, tag=f"qch{h}", name=f"qch{h}_{c}")
                        for h in range(GQ)]
                q_chunks[c] = q_ch

                pending_rope = []

                def flush_rope():
                    # rotate-half via PE permutation matmul, then rope on DVE:
                    # dst = (raw+b)*cos + (perm@raw + swap(b))*ss
                    for t, raw in pending_rope:
                        bias_col = bias_sb[:, t:t + 1]
                        bias_sw = bias_sb[:, NT + t:NT + t + 1]
                        ps_sw = pp_t.tile([128, CH], F32, tag="ps_t", name="ps_sw")
                        nc.tensor.matmul(ps_sw, permb, raw, start=True, stop=True)
                        sw = p1.tile([128, CH], BF16, tag="sw")
                        nc.vector.scalar_tensor_tensor(
                            out=sw, in0=ps_sw, scalar=bias_sw, in1=ss_c,
                            op0=mybir.AluOpType.add, op1=mybir.AluOpType.mult)
                        dst = q_ch[t] if t < GQ else kT_full[:, sl]
                        nc.vector.scalar_tensor_tensor(
                            out=dst, in0=raw, scalar=bias_col, in1=cos_c,
                            op0=mybir.AluOpType.add, op1=mybir.AluOpType.mult)
                        nc.vector.tensor_add(dst, dst, sw)
                    pending_rope.clear()

                # t order: q0, q1, k, q2, q3, v; each tile's rope flushes one
                # MM-group later so the PSUM evacuation copy is never waited on
                t_list = [0, 1, GQ, 2, 3, NT - 1]
                for n, t in enumerate(t_list):
                    ps = pp_pair.tile([128, 2, CH], F32, tag="pair", name="ps_p")
                    half = n % 2
                    for kt in range(KT):
                        nc.tensor.matmul(ps[:, half, :], w_sb[:, t, kt, :],
                                         x_c[:, kt, :],
                                         start=(kt == 0), stop=(kt == KT - 1))
                    if t == NT - 1:   # v (bias folded on host); transpose deferred
                        vT_c = p1.tile([128, CH], BF16, tag="vT_c")
                        nc.scalar.activation(out=vT_c, in_=ps[:, half, :],
                                             func=mybir.ActivationFunctionType.Copy)
                        for i4 in range(CH // 128):
                            deferred_v.append((c, vT_c, i4))
                    else:
                        raw = p1.tile([128, CH], BF16, tag="raw")
                        nc.scalar.activation(out=raw, in_=ps[:, half, :],
                                             func=mybir.ActivationFunctionType.Copy)
                        flush_rope()
                        pending_rope.append((t, raw))
                flush_rope()

            def emit_attention(j):
                """Emits the attention for chunk j with a 2-pair software
                pipeline across head boundaries."""
                flush_deferred_v()
                sl = bass.ds(j * CH, CH)
                live = live_per_j[j]
                masked = set(masked_per_j[j])
                # entry list: (i0, i1 or None, qlo, diag). Causal: plain tiles
                # paired full-width; diag tiles as singles restricted to their
                # live span [qlo, CH) with a [128,128] triangle mask on the
                # first 128 live columns.
                ents = []
                if causal:
                    plain = [i for i in live if i not in masked]
                    for n in range(0, len(plain), 2):
                        i1 = plain[n + 1] if n + 1 < len(plain) else None
                        ents.append((plain[n], i1, 0, False))
                    for i in sorted(masked):
                        ents.append((i, None, 128 * (i - 4 * j), True))
                else:
                    for n in range(0, len(live), 2):
                        i1 = live[n + 1] if n + 1 < len(live) else None
                        ents.append((live[n], i1, 0, False))

                outT_ch = [outp_pool.tile([128, CH], BF16, tag=f"outT{h}",
                                          name=f"outT{h}_{j}") for h in range(GQ)]
                q_ch = q_chunks[j]

                # global pipeline over (head, entry)
                work = [(h, e) for h in range(GQ) for e in ents]
                NP = len(work)
                attns = {}

                def emit_qk_exp(n):
                    h, (i0, i1, qlo, diag) = work[n]
                    qh = q_ch[h]
                    span = bass.ds(qlo, CH - qlo)
                    pr = pp_pair.tile([128, 2, CH], F32, tag="pair", name="pr")
                    nc.tensor.matmul(pr[:, 0, span],
                                     kT_full[:, bass.ds(i0 * 128, 128)],
                                     qh[:, span], start=True, stop=True)
                    if i1 is not None:
                        nc.tensor.matmul(pr[:, 1, :],
                                         kT_full[:, bass.ds(i1 * 128, 128)],
                                         qh, start=True, stop=True)
                    attn = att_pool.tile([128, 2, CH], BF16, tag="attn")
                    src = pr if i1 is not None else pr[:, 0:1, span]
                    dst = attn if i1 is not None else attn[:, 0:1, span]
                    nc.scalar.activation(out=dst, in_=src,
                                         func=mybir.ActivationFunctionType.Exp,
                                         scale=float(ATTN_SCALE))
                    if diag:
                        nc.vector.tensor_mul(attn[:, 0, bass.ds(qlo, 128)],
                                             attn[:, 0, bass.ds(qlo, 128)], tri)
                    elif (i0 in masked) or (i1 in masked):
                        mt = att_pool.tile([128, 2, CH], BF16, tag="m_tile",
                                           bufs=3, name=f"mt_{j}_{h}_{i0}")
                        nc.gpsimd.dma_start(out=mt[:, 0, :],
                                            in_=emaskT[bass.ds(i0 * 128, 128), sl])
                        if i1 is not None:
                            nc.gpsimd.dma_start(
                                out=mt[:, 1, :],
                                in_=emaskT[bass.ds(i1 * 128, 128), sl])
                        msl = attn if i1 is not None else attn[:, 0:1, :]
                        mm = mt if i1 is not None else mt[:, 0:1, :]
                        nc.vector.tensor_mul(msl, msl, mm)
                    attns[n] = attn

                def emit_av(n):
                    h, (i0, i1, qlo, diag) = work[n]
                    attn = attns.pop(n)
                    span = bass.ds(qlo, CH - qlo)
                    pidx = n % len(ents)
                    first, last = (pidx == 0), (pidx == len(ents) - 1)
                    ps_o = st_o[h]
                    ps_sum = st_sum[h]
                    nc.tensor.matmul(ps_o[:, span], v_nat[:, i0, :],
                                     attn[:, 0, span],
                                     start=first, stop=(last and i1 is None))
                    nc.tensor.matmul(ps_sum[:, span], ones_mat,
                                     attn[:, 0, span],
                                     start=first, stop=(last and i1 is None))
                    if i1 is not None:
                        nc.tensor.matmul(ps_o, v_nat[:, i1, :], attn[:, 1, :],
                                         start=False, stop=last)
                        nc.tensor.matmul(ps_sum, ones_mat, attn[:, 1, :],
                                         start=False, stop=last)
                    if last:
                        finalize(h)

                st_o, st_sum = {}, {}

                def start_head(h):
                    st_o[h] = pp_o.tile([128, CH], F32, tag="ps_o", name="ps_o")
                    st_sum[h] = pp_sum.tile([128, CH], F32, tag="ps_sum",
                                            name="ps_sum")

                def finalize(h):
                    # ps_sum already holds the denominator in every partition
                    recip = fin.tile([128, CH], F32, tag="recip")
                    nc.vector.reciprocal_approx_fast(out=recip, in_=st_sum[h])
                    nc.vector.tensor_mul(outT_ch[h], st_o[h], recip)

                PIPE = 2
                npairs = len(ents)
                for n in range(NP):
                    if n % npairs == 0:
                        start_head(work[n][0])
                    emit_qk_exp(n)
                    if n >= PIPE:
                        emit_av(n - PIPE)
                for n in range(max(0, NP - PIPE), NP):
                    emit_av(n)
                return outT_ch

            # ---- main fused loop: proj(c) -> o_proj(c-1) flush (outT finalize
            # gets proj-length slack) -> x(c+1) prefetch -> attention(c) ----
            pending_oproj = None
            for c in range(NCH):
                emit_proj(c)
                if pending_oproj is not None:
                    emit_oproj(pending_oproj)
                    pending_oproj = None
                first_att = True
                for j in range(NCH):
                    if need[j] == c:
                        if pending_oproj is not None:
                            emit_oproj(pending_oproj)
                            pending_oproj = None
                        if first_att:
                            # prefetch next chunk's x/cos/sin during attention
                            emit_prefetch(c)
                            first_att = False
                        pending_oproj = (j, emit_attention(j))
                if first_att:
                    emit_prefetch(c)

            if pending_oproj is not None:
                emit_oproj(pending_oproj, final=True)
                pending_oproj = None

    nc.finalize()
    return nc


_cache = {}


def _get_program(key, cls_grid, causal):
    if key not in _cache:
        _cache[key] = _build(cls_grid, causal)
    return _cache[key]


def _classify(em_t):
    """em_t: exp(mask).T [S, S] (ks, qs). Returns tuple-of-tuples class grid
    [NKS][NCH]."""
    grid = []
    for i in range(NKS):
        row = []
        for j in range(NCH):
            t = em_t[i * 128:(i + 1) * 128, j * CH:(j + 1) * CH]
            mx = t.max()
            mn = t.min()
            if mx == 0.0:
                row.append(SKIP)
            elif mn == 1.0 and mx == 1.0:
                row.append(PLAIN)
            else:
                row.append(MASKED)
        grid.append(tuple(row))
    return tuple(grid)


def _causal_grid():
    g = []
    for i in range(NKS):
        row = []
        for j in range(NCH):
            if i >= 4 * j + 4:
                row.append(SKIP)
            elif i >= 4 * j:
                row.append(MASKED)
            else:
                row.append(PLAIN)
        g.append(tuple(row))
    return tuple(g)


def _is_exact_causal(emaskT_b):
    """True iff exp(mask).T's diagonal band is exactly the causal 0/1
    pattern (off-band is covered by the grid comparison)."""
    p = np.arange(128)[:, None]
    for jj in range(NCH):
        for i in range(4 * jj, 4 * jj + 4):
            t = emaskT_b[i * 128:(i + 1) * 128, jj * CH:(jj + 1) * CH]
            d = i - 4 * jj
            q = np.arange(CH)[None, :]
            want = (p - q + 128 * d <= 0).astype(np.float32)
            if not np.array_equal(t, want):
                return False
    return True


def kernel(hidden_states, cos, sin, attention_mask,
           q_w, k_w, v_w, q_b, k_b, v_b,
           q_A, q_B, k_A, k_B, v_A, v_B, o_w):
    f32 = np.float32
    hidden_states = np.asarray(hidden_states, dtype=f32)
    cos = np.asarray(cos, dtype=f32)
    sin = np.asarray(sin, dtype=f32)
    mask = np.asarray(attention_mask, dtype=f32)[:, 0]  # [B, S, S]

    # host-side shared prep
    with np.errstate(under="ignore", over="ignore"):
        emask = np.exp(np.minimum(mask, 80.0))  # [B, S, S]; clamp avoids inf
    emaskT = [np.ascontiguousarray(emask[b].T) for b in range(B)]
    grids = [_classify(emaskT[b]) for b in range(B)]
    if grids[0] != grids[1]:
        grid = tuple(tuple(MASKED if (grids[0][i][j] != SKIP or grids[1][i][j] != SKIP)
                           else SKIP for j in range(NCH)) for i in range(NKS))
    else:
        grid = grids[0]
    for j in range(NCH):
        if all(grid[i][j] == SKIP for i in range(NKS)):
            grid = tuple(tuple(MASKED for _ in range(NCH)) for _ in range(NKS))
            break

    causal = (grid == _causal_grid()
              and all(_is_exact_causal(emaskT[b]) for b in range(B)))

    nc = _get_program((grid, causal), grid, causal)

    # x_pre[c, p, kt, s'] = x[b][c*CH+s', kt*128+p]
    xT = [np.ascontiguousarray(
        hidden_states[b].reshape(NCH, CH, KT, 128).transpose(0, 3, 2, 1)
        ).astype(NPBF16) for b in range(B)]
    cosT = [np.ascontiguousarray(cos[b].T).astype(NPBF16) for b in range(B)]
    ss = np.concatenate([-sin[:, :, :HD // 2], sin[:, :, HD // 2:]], axis=-1)
    ssT = [np.ascontiguousarray(ss[b].T).astype(NPBF16) for b in range(B)]
    emaskT16 = None

    # effective weights: W_eff[outdim, h] = W[outdim, h] + s*(A @ B).T[outdim, h]
    qw_eff = q_w + LORA_SCALE * (q_A @ q_B).T
    kw_eff = k_w + LORA_SCALE * (k_A @ k_B).T
    vw_eff = v_w + LORA_SCALE * (v_A @ v_B).T

    in_maps = []
    for c in range(NCORES):
        b, g = divmod(c, KVH)
        qsl = slice(QD * g, QD * (g + 1))
        ksl = slice(HD * g, HD * (g + 1))
        w_cat = np.concatenate([qw_eff[qsl], kw_eff[ksl], vw_eff[ksl]], axis=0)
        # w_pre[p, t, kt, o] = w_cat[t*128+o, kt*128+p]
        wT_c = w_cat.reshape(NT, 128, KT, 128).transpose(3, 0, 2, 1)
        # v bias handled on host: after softmax-normalization its contribution
        # to the output is the constant row o_w @ vb_o (added post-gather)
        bias_cat = np.concatenate([q_b[qsl], k_b[ksl],
                                   np.zeros(HD, f32)]).astype(f32)
        bias_cols = bias_cat.reshape(NT, 128).T  # [128, NT]
        swap_idx = np.concatenate([np.arange(64, 128), np.arange(0, 64)])
        biasT_c = np.ascontiguousarray(
            np.concatenate([bias_cols, bias_cols[swap_idx]], axis=1))  # [128, 2*NT]
        owT_c = o_w[:, qsl].T
        m = {
            "xT": xT[b],
            "wT": np.ascontiguousarray(wT_c).astype(NPBF16),
            "biasT": biasT_c,
            "cachetag": np.zeros((1, (K_TAG_INT % 97) + 1), f32),
            "cosT": cosT[b],
            "ssT": ssT[b],
            "owT": np.ascontiguousarray(owT_c).astype(NPBF16),
        }
        if not causal and any(grid[i][j] == MASKED for i in range(NKS) for j in range(NCH)):
            if emaskT16 is None:
                emaskT16 = [e.astype(NPBF16) for e in emaskT]
            m["emaskT"] = emaskT16[b]
        in_maps.append(m)

    res = run_bass_kernel_spmd(nc, in_maps, core_ids=list(range(NCORES)))
    outs = [np.asarray(r["out_p"], dtype=f32) for r in res.results]
    # v-bias contribution: softmax rows sum to 1, so the +v_b term passes
    # through attention unchanged and adds o_w @ vb_o to every output row
    vb_o = np.empty(NH * HD, f32)
    for g in range(KVH):
        vb_o[QD * g:QD * (g + 1)] = np.tile(v_b[HD * g:HD * (g + 1)], GQ)
    delta = (o_w.astype(f32) @ vb_o)[None, :]  # [1, H]
    full = np.empty((B, S, H), f32)
    for b in range(B):
        full[b] = outs[KVH * b]
        for g in range(1, KVH):
            full[b] += outs[KVH * b + g]
        full[b] += delta
    return full


# revision 62
# speedup vs baseline: 1.2148x; 1.0215x over previous
"""Trainium2 Bass kernel for LoRA-fused QKV + RoPE + GQA causal attention + o_proj.

Problem (hardcoded): B=2, S=2048, H=2048, NH=16, KVH=4, HD=128, R=16.

Sharding: 8 cores = batch(2) x kv-head-group(4). Core c handles batch b=c//4,
kv head g=c%4 (q heads 4g..4g+3). Each core computes its 4 heads' attention and
a partial o_proj ([S,H] partial over its 512 o-dims); host sums 4 partials per
batch.

v2 design vs the fp32r baseline:
- All matmuls in bf16 (1 cycle/row like fp32r, but FWL weight loads and half
  the DMA/SBUF). PSUM accumulation stays fp32. LoRA is folded into W on the
  host (W_eff = W^T + scale*A@B), biases are applied by the scalar engine
  during PSUM evacuation (activation bias=AP per-partition column).
- Everything stays in "transposed space": projections produce qT/kT/vT [d, s],
  scoresT [ks, qs] feeds AV directly, o_proj consumes outT [d, s] stationary.
- Softmax: no max-subtraction; exp on the scalar engine over PAIRS of score
  tiles (one ACTIVATE per 2 PSUM banks halves the ~310ns/instr overhead);
  column sums via an all-ones stationary matmul into PSUM; normalization on
  DVE (reciprocal_approx_fast + gpsimd partition_broadcast + multiply).
- Causal mask applied multiplicatively as exp(mask) with SKIP tiles dropped;
  the diagonal-tile patterns are generated on device as paired bf16 tiles.
- Single fused loop over 4 s-chunks; o_proj of chunk j flushes at the start of
  chunk j+1; attention is software-pipelined 2 score-pairs deep across head
  boundaries so PE never waits on exp.
- Weights resident in SBUF (loaded once); x/cos/sin double-buffered and
  prefetched a chunk ahead; initial DMAs split across queues so the first
  matmul starts ~2us in; output tiles stream back round-robin on 4 queues.
"""

import hashlib
import numpy as np
import ml_dtypes

import concourse.bass as bass
import concourse.mybir as mybir
import concourse.tile as tile
from concourse import bacc
from concourse.bass_utils import run_bass_kernel_spmd

B, S, H = 2, 2048, 2048
NH, KVH, HD = 16, 4, 128
R = 16
LORA_SCALE = 32.0 / 16.0
ATTN_SCALE = HD ** -0.5

NCORES = 8
GQ = NH // KVH          # 4 q heads per core
NT = GQ + 2             # 6 projection tiles: 4 q heads, 1 k, 1 v
QD = GQ * HD            # 512
CH = 512                # s-chunk width
NCH = S // CH           # 4 s-chunks
KT = H // 128           # 16 contraction k-tiles
NKS = S // 128          # 16 ks tiles
F32 = mybir.dt.float32
F32R = mybir.dt.float32r
BF16 = mybir.dt.bfloat16
NPBF16 = ml_dtypes.bfloat16

# tile classification codes (host-computed from exp(mask) tiles)
SKIP, PLAIN, MASKED = 0, 1, 2

# content tag: force a fresh NEFF cache key whenever this file changes
with open(__file__, "rb") as _f:
    KTAG = hashlib.sha1(_f.read()).hexdigest()[:10]
K_TAG_INT = int(KTAG, 16)


def _build(cls_grid, causal):
    """Build the SPMD program. cls_grid[i][j] in {SKIP, PLAIN, MASKED} for
    scoresT tile (ks_tile i, qs_chunk j). causal=True generates the diagonal
    mask tiles on device (no emaskT input)."""
    nc = bacc.Bacc("TRN2", target_bir_lowering=False)

    # host-packed for contiguous per-partition DMA:
    # x_pre[c, p, kt, s'] = x[b][s = c*CH+s', h = kt*128+p]  (bf16)
    xT = nc.dram_tensor("xT", [NCH, 128, KT, CH], BF16, kind="ExternalInput")
    # w_pre[p, t, kt, o] = w_eff[h = kt*128+p, t*128+o]  (bf16, LoRA folded)
    wT = nc.dram_tensor("wT", [128, NT, KT, 128], BF16, kind="ExternalInput")
    # [:, 0:NT] plain bias columns; [:, NT:2*NT] partition-swapped (rotate-half)
    biasT = nc.dram_tensor("biasT", [128, 2 * NT], F32, kind="ExternalInput")
    # cache-buster: the PJRT NEFF cache hashes the HLO minus backend_config
    DL = (K_TAG_INT % 97) + 1
    dummy = nc.dram_tensor("cachetag", [1, DL], F32, kind="ExternalInput")
    cosT = nc.dram_tensor("cosT", [HD, S], BF16, kind="ExternalInput")
    ssT = nc.dram_tensor("ssT", [HD, S], BF16, kind="ExternalInput")
    any_masked = any(cls_grid[i][j] == MASKED for i in range(NKS) for j in range(NCH))
    emaskT = None
    if not causal and any_masked:
        emaskT = nc.dram_tensor("emaskT", [S, S], BF16, kind="ExternalInput")
    owT = nc.dram_tensor("owT", [QD, H], BF16, kind="ExternalInput")
    out_p = nc.dram_tensor("out_p", [S, H], BF16, kind="ExternalOutput")

    live_per_j = [[i for i in range(NKS) if cls_grid[i][jj] != SKIP]
                  for jj in range(NCH)]
    masked_per_j = [[i for i in range(NKS) if cls_grid[i][jj] == MASKED]
                    for jj in range(NCH)]
    need = [max(jj, max(live_per_j[jj]) // (CH // 128)) for jj in range(NCH)]
    QCH_BUFS = max(2, max(need[jj] - jj for jj in range(NCH)) + 1)

    with tile.TileContext(nc) as tc:
        from concourse.masks import make_identity
        with tc.tile_pool(name="consts", bufs=1) as consts, \
             tc.tile_pool(name="persist", bufs=1) as persist, \
             tc.tile_pool(name="qch", bufs=QCH_BUFS) as qch_pool, \
             tc.tile_pool(name="outp", bufs=2) as outp_pool, \
             tc.tile_pool(name="p1", bufs=5) as p1, \
             tc.tile_pool(name="xch", bufs=2) as xch_pool, \
             tc.tile_pool(name="att", bufs=3) as att_pool, \
             tc.tile_pool(name="stgp", bufs=8) as stgp, \
             tc.tile_pool(name="fin", bufs=2) as fin, \
             tc.tile_pool(name="pp_pair", bufs=2, space="PSUM") as pp_pair, \
             tc.tile_pool(name="pp_o", bufs=2, space="PSUM") as pp_o, \
             tc.tile_pool(name="pp_sum", bufs=1, space="PSUM") as pp_sum, \
             tc.tile_pool(name="pp_t", bufs=1, space="PSUM") as pp_t:

            # ---- chunk-0 inputs first: x pieces split across queues ----
            x_tiles = {}

            def emit_x_dma(c, fine=False):
                x_c = xch_pool.tile([128, KT, CH], BF16, tag="x_c", name=f"x_{c}")
                x_tiles[c] = x_c
                if fine:
                    # 16 single-kt pieces: first matmul starts after 0.13MB
                    for kt in range(KT):
                        q = nc.sync if kt % 2 == 0 else nc.gpsimd
                        q.dma_start(out=x_c[:, bass.ds(kt, 1), :],
                                    in_=xT[c, :, bass.ds(kt, 1), :])
                else:
                    qs = [nc.sync, nc.gpsimd, nc.sync, nc.gpsimd]
                    for kq in range(4):
                        qs[kq].dma_start(out=x_c[:, bass.ds(kq * 4, 4), :],
                                         in_=xT[c, :, bass.ds(kq * 4, 4), :])

            cs_tiles = {}

            def emit_cs_dma(c):
                sl = bass.ds(c * CH, CH)
                cos_c = xch_pool.tile([128, CH], BF16, tag="cos_c", name=f"cos_{c}")
                nc.sync.dma_start(out=cos_c, in_=cosT[:, sl])
                ss_c = xch_pool.tile([128, CH], BF16, tag="ss_c", name=f"ss_{c}")
                nc.sync.dma_start(out=ss_c, in_=ssT[:, sl])
                cs_tiles[c] = (cos_c, ss_c)

            # ---- weights resident, all on the scalar queue in t-need order;
            # x chunk 0 finely split on sync/gpsimd in parallel ----
            w_sb = persist.tile([128, NT, KT, 128], BF16, tag="w_sb")
            # t0 in 4 fine pieces so the first matmul starts after ~0.16MB
            for kq in range(4):
                nc.scalar.dma_start(out=w_sb[:, 0, bass.ds(kq * 4, 4), :],
                                    in_=wT[:, 0, bass.ds(kq * 4, 4), :])
            emit_x_dma(0, fine=True)
            for t in [1, GQ, 2, 3, NT - 1]:   # proj t-need order
                nc.scalar.dma_start(out=w_sb[:, t, :, :], in_=wT[:, t, :, :])
            emit_cs_dma(0)
            bias_sb = consts.tile([128, 2 * NT], F32, tag="bias_sb")
            nc.gpsimd.dma_start(out=bias_sb, in_=biasT[:, :])
            dummy_sb = consts.tile([1, 128], F32, tag="dummy_sb")
            nc.gpsimd.dma_start(out=dummy_sb[:, 0:DL], in_=dummy[:, :])

            # ---- small constants ----
            # full 128-col all-ones stationary: the denominator matmul then
            # broadcasts the column sums across all 128 PSUM partitions (no
            # gpsimd partition_broadcast needed) and keeps LDWEIGHTS
            # pull-ahead working (no col_grp restriction)
            ones_mat = consts.tile([128, 128], BF16, tag="ones_mat")
            nc.vector.memset(ones_mat, 1.0)
            ident_f = consts.tile([128, 128], F32, tag="ident_f")
            make_identity(nc, ident_f)
            identb = consts.tile([128, 128], BF16, tag="identb")
            nc.vector.tensor_copy(out=identb, in_=ident_f)

            # half-rotation permutation: perm[p, q] = 1 iff q == (p+64)%128.
            # Used as a matmul stationary to compute rotate-half on the PE
            # (swap via DMA would cross partitions on a busy queue).
            perm_f = consts.tile([128, 128], F32, tag="perm_f")
            nc.gpsimd.memset(perm_f, 0.0)
            for base in (64, -64):
                nc.gpsimd.affine_select(
                    out=perm_f, in_=perm_f,
                    compare_op=mybir.AluOpType.not_equal,
                    fill=1.0,
                    base=base,
                    channel_multiplier=1,
                    pattern=[[-1, 128]],
                )
            permb = consts.tile([128, 128], BF16, tag="permb")
            nc.vector.tensor_copy(out=permb, in_=perm_f)

            # causal: single [128,128] lower-triangle mask; diag tiles are
            # processed as singles restricted to their live column span
            # [128*d, CH), where only the first 128 columns are triangular
            tri = None
            if causal:
                scratch = consts.tile([128, 128], F32, tag="tri_scratch")
                nc.gpsimd.memset(scratch, 0.0)
                nc.gpsimd.affine_select(
                    out=scratch, in_=scratch,
                    compare_op=mybir.AluOpType.is_gt,
                    fill=1.0,
                    base=0,
                    channel_multiplier=1,
                    pattern=[[-1, 128]],
                )
                tri = consts.tile([128, 128], BF16, tag="tri")
                nc.vector.tensor_copy(out=tri, in_=scratch)

            # ow and next-chunk inputs are prefetched lazily (first attention)
            # so they don't steal startup fabric bandwidth from x(0)/w
            ow_sb = persist.tile([128, GQ, H], BF16, tag="ow_sb")
            ow_done = [False]

            def emit_prefetch(c):
                if c + 1 < NCH:
                    emit_x_dma(c + 1)
                    emit_cs_dma(c + 1)
                if not ow_done[0]:
                    ow_done[0] = True
                    nc.gpsimd.dma_start(
                        out=ow_sb, in_=owT.rearrange("(g p) n -> p g n", p=128))

            # ---- persistent tiles ----
            kT_full = persist.tile([128, S], BF16, tag="kT_full")
            v_nat = persist.tile([128, NKS, 128], BF16, tag="v_nat")  # [ks, tile, d]

            out_dma_q = [nc.sync, nc.gpsimd]
            out_dma_n = [0]

            def emit_oproj(args, final=False):
                cc, outT_ch = args
                qlist = [nc.sync, nc.gpsimd, nc.scalar] if final else [nc.gpsimd, nc.scalar]
                for st4 in range(CH // 128):
                    ssl = bass.ds(st4 * 128, 128)
                    dsl = bass.ds((cc * (CH // 128) + st4) * 128, 128)
                    for nch in range(NCH):
                        pop_deferred_v()
                        nsl = bass.ds(nch * CH, CH)
                        ps3 = pp_pair.tile([128, 2, CH], F32, tag="pair", name="ps3")
                        g = st4 * NCH + nch
                        half = g % 2
                        for h in range(GQ):
                            nc.tensor.matmul(ps3[:, half, :], outT_ch[h][:, ssl],
                                             ow_sb[:, h, nsl],
                                             start=(h == 0), stop=(h == GQ - 1))
                        stg = stgp.tile([128, CH], BF16, tag="stg")
                        if g % 2 == 0:
                            nc.vector.tensor_copy(out=stg, in_=ps3[:, half, :])
                        else:
                            nc.scalar.activation(out=stg, in_=ps3[:, half, :],
                                                 func=mybir.ActivationFunctionType.Copy)
                        q = qlist[out_dma_n[0] % len(qlist)]
                        out_dma_n[0] += 1
                        q.dma_start(out=out_p[dsl, nsl], in_=stg)

            q_chunks = {}
            deferred_v = []

            def pop_deferred_v():
                """Emit ONE pending v transpose (or nothing)."""
                if not deferred_v:
                    return
                c, vT_c, i4 = deferred_v.pop(0)
                i = c * (CH // 128) + i4
                ps_t = pp_t.tile([128, 128], BF16, tag="ps_t", name="ps_t")
                nc.tensor.transpose(ps_t, vT_c[:, bass.ds(i4 * 128, 128)], identb)
                nc.vector.tensor_copy(out=v_nat[:, i, :], in_=ps_t)

            def flush_deferred_v():
                while deferred_v:
                    pop_deferred_v()

            def emit_proj(c):
                x_c = x_tiles[c]
                cos_c, ss_c = cs_tiles[c]
                sl = bass.ds(c * CH, CH)

                q_ch = [qch_pool.tile([128, CH], BF16, tag=f"qch{h}", name=f"qch{h}_{c}")
                        for h in range(GQ)]
                q_chunks[c] = q_ch

                pending_rope = []

                def flush_rope():
                    # rotate-half via PE permutation matmul, then rope on DVE:
                    # dst = (raw+b)*cos + (perm@raw + swap(b))*ss
                    for t, raw in pending_rope:
                        bias_col = bias_sb[:, t:t + 1]
                        bias_sw = bias_sb[:, NT + t:NT + t + 1]
                        ps_sw = pp_t.tile([128, CH], F32, tag="ps_t", name="ps_sw")
                        nc.tensor.matmul(ps_sw, permb, raw, start=True, stop=True)
                        sw = p1.tile([128, CH], BF16, tag="sw")
                        nc.vector.scalar_tensor_tensor(
                            out=sw, in0=ps_sw, scalar=bias_sw, in1=ss_c,
                            op0=mybir.AluOpType.add, op1=mybir.AluOpType.mult)
                        dst = q_ch[t] if t < GQ else kT_full[:, sl]
                        nc.vector.scalar_tensor_tensor(
                            out=dst, in0=raw, scalar=bias_col, in1=cos_c,
                            op0=mybir.AluOpType.add, op1=mybir.AluOpType.mult)
                        nc.vector.tensor_add(dst, dst, sw)
                    pending_rope.clear()

                # t order: q0, q1, k, q2, q3, v; each tile's rope flushes one
                # MM-group later so the PSUM evacuation copy is never waited on
                t_list = [0, 1, GQ, 2, 3, NT - 1]
                for n, t in enumerate(t_list):
                    ps = pp_pair.tile([128, 2, CH], F32, tag="pair", name="ps_p")
                    half = n % 2
                    for kt in range(KT):
                        nc.tensor.matmul(ps[:, half, :], w_sb[:, t, kt, :],
                                         x_c[:, kt, :],
                                         start=(kt == 0), stop=(kt == KT - 1))
                    if t == NT - 1:   # v (bias folded on host); transpose deferred
                        vT_c = p1.tile([128, CH], BF16, tag="vT_c")
                        nc.scalar.activation(out=vT_c, in_=ps[:, half, :],
                                             func=mybir.ActivationFunctionType.Copy)
                        for i4 in range(CH // 128):
                            deferred_v.append((c, vT_c, i4))
                    else:
                        raw = p1.tile([128, CH], BF16, tag="raw")
                        nc.scalar.activation(out=raw, in_=ps[:, half, :],
                                             func=mybir.ActivationFunctionType.Copy)
                        flush_rope()
                        pending_rope.append((t, raw))
                flush_rope()

            def emit_attention(j):
                """Emits the attention for chunk j with a 2-pair software
                pipeline across head boundaries."""
                flush_deferred_v()
                sl = bass.ds(j * CH, CH)
                live = live_per_j[j]
                masked = set(masked_per_j[j])
                # entry list: (i0, i1 or None, qlo, diag). Causal: plain tiles
                # paired full-width; diag tiles as singles restricted to their
                # live span [qlo, CH) with a [128,128] triangle mask on the
                # first 128 live columns.
                ents = []   # (i0, i1|None, s0, s1|None, diag)
                if causal:
                    plain = [i for i in live if i not in masked]
                    for n in range(0, len(plain), 2):
                        i1 = plain[n + 1] if n + 1 < len(plain) else None
                        ents.append((plain[n], i1, 0, 0 if i1 is not None else None,
                                     False))
                    dg = sorted(masked)
                    for n in range(0, len(dg), 2):
                        i0 = dg[n]
                        i1 = dg[n + 1] if n + 1 < len(dg) else None
                        s0 = 128 * (i0 - 4 * j)
                        s1 = 128 * (i1 - 4 * j) if i1 is not None else None
                        ents.append((i0, i1, s0, s1, True))
                else:
                    for n in range(0, len(live), 2):
                        i1 = live[n + 1] if n + 1 < len(live) else None
                        ents.append((live[n], i1, 0, 0 if i1 is not None else None,
                                     False))

                outT_ch = [outp_pool.tile([128, CH], BF16, tag=f"outT{h}",
                                          name=f"outT{h}_{j}") for h in range(GQ)]
                q_ch = q_chunks[j]

                # global pipeline over (head, entry)
                work = [(h, e) for h in range(GQ) for e in ents]
                NP = len(work)
                attns = {}

                def emit_qk_exp(n):
                    h, (i0, i1, s0, s1, diag) = work[n]
                    qh = q_ch[h]
                    sp0 = bass.ds(s0, CH - s0)
                    pr = pp_pair.tile([128, 2, CH], F32, tag="pair", name="pr")
                    nc.tensor.matmul(pr[:, 0, sp0],
                                     kT_full[:, bass.ds(i0 * 128, 128)],
                                     qh[:, sp0], start=True, stop=True)
                    if i1 is not None:
                        sp1 = bass.ds(s1, CH - s1)
                        nc.tensor.matmul(pr[:, 1, sp1],
                                         kT_full[:, bass.ds(i1 * 128, 128)],
                                         qh[:, sp1], start=True, stop=True)
                    attn = att_pool.tile([128, 2, CH], BF16, tag="attn")
                    # one rectangular exp over both halves from s0 (s0 <= s1;
                    # half1's [s0,s1) region is garbage but is never read)
                    src = pr[:, :, sp0] if i1 is not None else pr[:, 0:1, sp0]
                    dst = attn[:, :, sp0] if i1 is not None else attn[:, 0:1, sp0]
                    nc.scalar.activation(out=dst, in_=src,
                                         func=mybir.ActivationFunctionType.Exp,
                                         scale=float(ATTN_SCALE))
                    if diag:
                        nc.vector.tensor_mul(attn[:, 0, bass.ds(s0, 128)],
                                             attn[:, 0, bass.ds(s0, 128)], tri)
                        if i1 is not None:
                            nc.vector.tensor_mul(attn[:, 1, bass.ds(s1, 128)],
                                                 attn[:, 1, bass.ds(s1, 128)], tri)
                    elif (i0 in masked) or (i1 in masked):
                        mt = att_pool.tile([128, 2, CH], BF16, tag="m_tile",
                                           bufs=3, name=f"mt_{j}_{h}_{i0}")
                        nc.gpsimd.dma_start(out=mt[:, 0, :],
                                            in_=emaskT[bass.ds(i0 * 128, 128), sl])
                        if i1 is not None:
                            nc.gpsimd.dma_start(
                                out=mt[:, 1, :],
                                in_=emaskT[bass.ds(i1 * 128, 128), sl])
                        msl = attn if i1 is not None else attn[:, 0:1, :]
                        mm = mt if i1 is not None else mt[:, 0:1, :]
                        nc.vector.tensor_mul(msl, msl, mm)
                    attns[n] = attn

                def emit_av(n):
                    h, (i0, i1, s0, s1, diag) = work[n]
                    attn = attns.pop(n)
                    sp0 = bass.ds(s0, CH - s0)
                    pidx = n % len(ents)
                    first, last = (pidx == 0), (pidx == len(ents) - 1)
                    ps_o = st_o[h]
                    ps_sum = st_sum[h]
                    nc.tensor.matmul(ps_o[:, sp0], v_nat[:, i0, :],
                                     attn[:, 0, sp0],
                                     start=first, stop=(last and i1 is None))
                    nc.tensor.matmul(ps_sum[:, sp0], ones_mat,
                                     attn[:, 0, sp0],
                                     start=first, stop=(last and i1 is None))
                    if i1 is not None:
                        sp1 = bass.ds(s1, CH - s1)
                        nc.tensor.matmul(ps_o[:, sp1], v_nat[:, i1, :],
                                         attn[:, 1, sp1],
                                         start=False, stop=last)
                        nc.tensor.matmul(ps_sum[:, sp1], ones_mat,
                                         attn[:, 1, sp1],
                                         start=False, stop=last)
                    if last:
                        finalize(h)

                st_o, st_sum = {}, {}

                def start_head(h):
                    st_o[h] = pp_o.tile([128, CH], F32, tag="ps_o", name="ps_o")
                    st_sum[h] = pp_sum.tile([128, CH], F32, tag="ps_sum",
                                            name="ps_sum")

                def finalize(h):
                    # ps_sum already holds the denominator in every partition
                    recip = fin.tile([128, CH], F32, tag="recip")
                    nc.vector.reciprocal_approx_fast(out=recip, in_=st_sum[h])
                    nc.vector.tensor_mul(outT_ch[h], st_o[h], recip)

                PIPE = 2
                npairs = len(ents)
                for n in range(NP):
                    if n % npairs == 0:
                        start_head(work[n][0])
                    emit_qk_exp(n)
                    if n >= PIPE:
                        emit_av(n - PIPE)
                for n in range(max(0, NP - PIPE), NP):
                    emit_av(n)
                return outT_ch

            # ---- main fused loop: proj(c) -> o_proj(c-1) flush (outT finalize
            # gets proj-length slack) -> x(c+1) prefetch -> attention(c) ----
            pending_oproj = None
            for c in range(NCH):
                emit_proj(c)
                if pending_oproj is not None:
                    emit_oproj(pending_oproj)
                    pending_oproj = None
                first_att = True
                for j in range(NCH):
                    if need[j] == c:
                        if pending_oproj is not None:
                            emit_oproj(pending_oproj)
                            pending_oproj = None
                        if first_att:
                            # prefetch next chunk's x/cos/sin during attention
                            emit_prefetch(c)
                            first_att = False
                        pending_oproj = (j, emit_attention(j))
                if first_att:
                    emit_prefetch(c)

            if pending_oproj is not None:
                emit_oproj(pending_oproj, final=True)
                pending_oproj = None

    nc.finalize()
    return nc


_cache = {}


def _get_program(key, cls_grid, causal):
    if key not in _cache:
        _cache[key] = _build(cls_grid, causal)
    return _cache[key]


def _classify(em_t):
    """em_t: exp(mask).T [S, S] (ks, qs). Returns tuple-of-tuples class grid
    [NKS][NCH]."""
    grid = []
    for i in range(NKS):
        row = []
        for j in range(NCH):
            t = em_t[i * 128:(i + 1) * 128, j * CH:(j + 1) * CH]
            mx = t.max()
            mn = t.min()
            if mx == 0.0:
                row.append(SKIP)
            elif mn == 1.0 and mx == 1.0:
                row.append(PLAIN)
            else:
                row.append(MASKED)
        grid.append(tuple(row))
    return tuple(grid)


def _causal_grid():
    g = []
    for i in range(NKS):
        row = []
        for j in range(NCH):
            if i >= 4 * j + 4:
                row.append(SKIP)
            elif i >= 4 * j:
                row.append(MASKED)
            else:
                row.append(PLAIN)
        g.append(tuple(row))
    return tuple(g)


def _is_exact_causal(emaskT_b):
    """True iff exp(mask).T's diagonal band is exactly the causal 0/1
    pattern (off-band is covered by the grid comparison)."""
    p = np.arange(128)[:, None]
    for jj in range(NCH):
        for i in range(4 * jj, 4 * jj + 4):
            t = emaskT_b[i * 128:(i + 1) * 128, jj * CH:(jj + 1) * CH]
            d = i - 4 * jj
            q = np.arange(CH)[None, :]
            want = (p - q + 128 * d <= 0).astype(np.float32)
            if not np.array_equal(t, want):
                return False
    return True


def kernel(hidden_states, cos, sin, attention_mask,
           q_w, k_w, v_w, q_b, k_b, v_b,
           q_A, q_B, k_A, k_B, v_A, v_B, o_w):
    f32 = np.float32
    hidden_states = np.asarray(hidden_states, dtype=f32)
    cos = np.asarray(cos, dtype=f32)
    sin = np.asarray(sin, dtype=f32)
    mask = np.asarray(attention_mask, dtype=f32)[:, 0]  # [B, S, S]

    # host-side shared prep
    with np.errstate(under="ignore", over="ignore"):
        emask = np.exp(np.minimum(mask, 80.0))  # [B, S, S]; clamp avoids inf
    emaskT = [np.ascontiguousarray(emask[b].T) for b in range(B)]
    grids = [_classify(emaskT[b]) for b in range(B)]
    if grids[0] != grids[1]:
        grid = tuple(tuple(MASKED if (grids[0][i][j] != SKIP or grids[1][i][j] != SKIP)
                           else SKIP for j in range(NCH)) for i in range(NKS))
    else:
        grid = grids[0]
    for j in range(NCH):
        if all(grid[i][j] == SKIP for i in range(NKS)):
            grid = tuple(tuple(MASKED for _ in range(NCH)) for _ in range(NKS))
            break

    causal = (grid == _causal_grid()
              and all(_is_exact_causal(emaskT[b]) for b in range(B)))

    nc = _get_program((grid, causal), grid, causal)

    # x_pre[c, p, kt, s'] = x[b][c*CH+s', kt*128+p]
    xT = [np.ascontiguousarray(
        hidden_states[b].reshape(NCH, CH, KT, 128).transpose(0, 3, 2, 1)
        ).astype(NPBF16) for b in range(B)]
    cosT = [np.ascontiguousarray(cos[b].T).astype(NPBF16) for b in range(B)]
    ss = np.concatenate([-sin[:, :, :HD // 2], sin[:, :, HD // 2:]], axis=-1)
    ssT = [np.ascontiguousarray(ss[b].T).astype(NPBF16) for b in range(B)]
    emaskT16 = None

    # effective weights: W_eff[outdim, h] = W[outdim, h] + s*(A @ B).T[outdim, h]
    qw_eff = q_w + LORA_SCALE * (q_A @ q_B).T
    kw_eff = k_w + LORA_SCALE * (k_A @ k_B).T
    vw_eff = v_w + LORA_SCALE * (v_A @ v_B).T

    in_maps = []
    for c in range(NCORES):
        b, g = divmod(c, KVH)
        qsl = slice(QD * g, QD * (g + 1))
        ksl = slice(HD * g, HD * (g + 1))
        w_cat = np.concatenate([qw_eff[qsl], kw_eff[ksl], vw_eff[ksl]], axis=0)
        # w_pre[p, t, kt, o] = w_cat[t*128+o, kt*128+p]
        wT_c = w_cat.reshape(NT, 128, KT, 128).transpose(3, 0, 2, 1)
        # v bias handled on host: after softmax-normalization its contribution
        # to the output is the constant row o_w @ vb_o (added post-gather)
        bias_cat = np.concatenate([q_b[qsl], k_b[ksl],
                                   np.zeros(HD, f32)]).astype(f32)
        bias_cols = bias_cat.reshape(NT, 128).T  # [128, NT]
        swap_idx = np.concatenate([np.arange(64, 128), np.arange(0, 64)])
        biasT_c = np.ascontiguousarray(
            np.concatenate([bias_cols, bias_cols[swap_idx]], axis=1))  # [128, 2*NT]
        owT_c = o_w[:, qsl].T
        m = {
            "xT": xT[b],
            "wT": np.ascontiguousarray(wT_c).astype(NPBF16),
            "biasT": biasT_c,
            "cachetag": np.zeros((1, (K_TAG_INT % 97) + 1), f32),
            "cosT": cosT[b],
            "ssT": ssT[b],
            "owT": np.ascontiguousarray(owT_c).astype(NPBF16),
        }
        if not causal and any(grid[i][j] == MASKED for i in range(NKS) for j in range(NCH)):
            if emaskT16 is None:
                emaskT16 = [e.astype(NPBF16) for e in emaskT]
            m["emaskT"] = emaskT16[b]
        in_maps.append(m)

    res = run_bass_kernel_spmd(nc, in_maps, core_ids=list(range(NCORES)))
    outs = [np.asarray(r["out_p"], dtype=f32) for r in res.results]
    # v-bias contribution: softmax rows sum to 1, so the +v_b term passes
    # through attention unchanged and adds o_w @ vb_o to every output row
    vb_o = np.empty(NH * HD, f32)
    for g in range(KVH):
        vb_o[QD * g:QD * (g + 1)] = np.tile(v_b[HD * g:HD * (g + 1)], GQ)
    delta = (o_w.astype(f32) @ vb_o)[None, :]  # [1, H]
    full = np.empty((B, S, H), f32)
    for b in range(B):
        full[b] = outs[KVH * b]
        for g in range(1, KVH):
            full[b] += outs[KVH * b + g]
        full[b] += delta
    return full
